# revision 17
# baseline (speedup 1.0000x reference)
"""ArchetypalNeuralMemory on 8 TRN2 NeuronCores (Bass/Tile).

Strategy (sharding_hint: data-parallel over B, replicate fast weights):
  - token sharding: core c owns batch b=c//2, parity p=c%2 -> tokens
    x[b, p::2, :]  (2048 tokens; 32 rows of every one of the 64 chunks).
  - projections (k,kT,v) computed on the local shard in 4 token-groups;
    each group is packed into one buffer and all-gathered so the gathers
    overlap later projection compute and the early scan steps.
  - the 64-step fast-weight scan is inherently serial, so it is
    REPLICATED on every core; only retrieval + output proj are sharded.
  - scalar engine runs ONLY Sigmoid in the scan (no ACT table reloads);
    silu and silu' are built from sigmoid on the vector engine.
  - momentum update is a vector scalar_tensor_tensor reading the grad
    PSUM directly (no eta*m matmul, no psum->sbuf momentum copy).
  - fp32 weight masters updated on the (otherwise idle) gpsimd engine;
    the bf16 shadows are produced first so the critical path never
    waits on the fp32 update.
"""

import sys

if "/opt/trn_rl_repo" not in sys.path:
    sys.path.insert(0, "/opt/trn_rl_repo")

import numpy as np

B, S, D = 4, 4096, 512
C = 64            # chunk length
NSTEP = 64        # chunks
NCORE = 8
TOK = 2048        # tokens per core
NT = TOK // 128   # 16 row tiles per core
NGRP = 4          # projection/AG groups
TPG = NT // NGRP  # tiles per group
LR, MOM, DEC = 0.1, 0.9, 0.01
EPS_RMS = 1.1920929e-07
INV_N = 2.0 / (B * C * D)

_BUILT = {}


def _build(n_steps=NSTEP):
    import concourse.bacc as bacc
    import concourse.mybir as mybir
    import concourse.tile as tile
    from contextlib import ExitStack
    import os as _os

    F32 = mybir.dt.float32
    BF16 = mybir.dt.bfloat16
    AF = mybir.ActivationFunctionType
    ALU = mybir.AluOpType
    AX = mybir.AxisListType

    nc = bacc.Bacc("TRN2", target_bir_lowering=False)
    P = nc.declare_dram_parameter

    xs_d = P("xs", [TOK, D], F32, isOutput=False)
    mb_d = P("Mb", [D, D], BF16, isOutput=False)        # gs-scaled M[b], [d,e]
    wkT_d = P("WkT", [D, D], BF16, isOutput=False)       # [e, e']
    wvT_d = P("WvT", [D, D], BF16, isOutput=False)       # gs-folded, [d, e]
    wqT_d = P("WqT", [D, D], BF16, isOutput=False)       # gr-folded, [d, e]
    woT_d = P("WoutT", [D, D], BF16, isOutput=False)     # [d, e]
    wgT_d = [P(n, [D, D], BF16, isOutput=False) for n in ("WgdT", "WglT", "WgmT")]
    bg_d = [P(n, [D, 1], F32, isOutput=False) for n in ("bgd", "bgl", "bgm")]
    w0t32_d = P("W0T32", [D, D], F32, isOutput=False)    # mem_W[0].T
    w0t16_d = P("W0T16", [D, D], BF16, isOutput=False)
    w1n32_d = P("W1n32", [D, D], F32, isOutput=False)    # mem_W[1]
    w1n16_d = P("W1n16", [D, D], BF16, isOutput=False)
    w1t16_d = P("W1T16", [D, D], BF16, isOutput=False)   # mem_W[1].T
    eye16_d = P("EYE16", [128, 128], BF16, isOutput=False)
    eye32_d = P("EYE32", [128, 128], F32, isOutput=False)
    out_d = P("out", [TOK, D], F32, isOutput=True)

    with tile.TileContext(nc) as tc, ExitStack() as ctx:
        dram = ctx.enter_context(tc.tile_pool(name="dram", bufs=1, space="DRAM"))
        ps_mm = ctx.enter_context(tc.tile_pool(name="ps_mm", bufs=4, space="PSUM"))
        ps_gr = ctx.enter_context(tc.tile_pool(name="ps_gr", bufs=2, space="PSUM"))
        ps_tr = ctx.enter_context(tc.tile_pool(name="ps_tr", bufs=2, space="PSUM"))
        pers = ctx.enter_context(tc.tile_pool(name="pers", bufs=1))
        pa = ctx.enter_context(tc.tile_pool(name="pa", bufs=1))
        par = ctx.enter_context(tc.tile_pool(name="par", bufs=2))
        parg = ctx.enter_context(tc.tile_pool(name="parg", bufs=1))
        rot = ctx.enter_context(tc.tile_pool(name="rot", bufs=2))
        rot3 = ctx.enter_context(tc.tile_pool(name="rot3", bufs=3))

        def dmaload(pool, shape, dt, src, tag):
            t = pool.tile(shape, dt, tag=tag, name=tag)
            nc.sync.dma_start(t[:], src)
            return t

        # ---------- persistent state ----------
        eye16 = dmaload(pers, [128, 128], BF16, eye16_d[:], "eye16")
        eye32 = dmaload(pers, [128, 128], F32, eye32_d[:], "eye32")
        w0t32 = [dmaload(pers, [128, D], F32, w0t32_d[128 * i : 128 * (i + 1), :], f"w0t32_{i}") for i in range(4)]
        w1n32 = [dmaload(pers, [128, D], F32, w1n32_d[128 * i : 128 * (i + 1), :], f"w1n32_{i}") for i in range(4)]
        w0t16 = [dmaload(pers, [128, D], BF16, w0t16_d[128 * i : 128 * (i + 1), :], f"w0t16_{i}") for i in range(4)]
        w1n16 = [dmaload(pers, [128, D], BF16, w1n16_d[128 * i : 128 * (i + 1), :], f"w1n16_{i}") for i in range(4)]
        w1t16 = [dmaload(pers, [128, D], BF16, w1t16_d[128 * i : 128 * (i + 1), :], f"w1t16_{i}") for i in range(4)]
        avc = pers.tile([128, NSTEP], F32, tag="avc", name="avc")
        evc = pers.tile([128, NSTEP], F32, tag="evc", name="evc")
        cvc = pers.tile([128, NSTEP], F32, tag="cvc", name="cvc")
        eps_t = pers.tile([128, 1], F32, tag="eps", name="eps")
        nc.vector.memset(eps_t[:], EPS_RMS)
        m0 = [pers.tile([128, D], BF16, tag=f"m0_{i}", name=f"m0_{i}") for i in range(4)]
        m1 = [pers.tile([128, D], BF16, tag=f"m1_{i}", name=f"m1_{i}") for i in range(4)]
        for i in range(4):
            nc.vector.memset(m0[i][:], 0.0)
            nc.vector.memset(m1[i][:], 0.0)
        xtT16 = [pers.tile([128, D], BF16, tag=f"xtT{rt}", name=f"xtT{rt}") for rt in range(NT)]
        ss_all = pers.tile([128, NT], F32, tag="ss_all", name="ss_all")
        rinv_all = pers.tile([128, NT], F32, tag="rinv_all", name="rinv_all")
        cmp_ = [pers.tile([128, NSTEP], F32, tag=f"cmp{i}", name=f"cmp{i}") for i in range(4)]

        # dram buffers
        qT_d = dram.tile([D, TOK], BF16)
        ret_d = dram.tile([TOK, D], BF16)
        cm_sh = dram.tile([D, NSTEP], F32)
        cmG = dram.tile([D * NCORE, NSTEP], F32, addr_space="Shared")
        # packed per-group projection buffer: rows 0-511 kT, 512-1023 k, 1024-1535 v
        pkg = [dram.tile([3 * D, D], BF16, tag=f"pkg{g}", name=f"pkg{g}") for g in range(NGRP)]
        pkgG = [dram.tile([3 * D * NCORE, D], BF16, addr_space="Shared", tag=f"pkgG{g}", name=f"pkgG{g}")
                for g in range(NGRP)]
        gbounce = dram.tile([1, 3 * NSTEP], F32)

        skip_ag = _os.environ.get("KERNEL_SKIP_AG") == "1"
        skip_proj = _os.environ.get("KERNEL_SKIP_PROJ") == "1"
        skip_c = _os.environ.get("KERNEL_SKIP_PHASEC") == "1"

        def allgather(src, dst):
            if skip_ag:
                nc.gpsimd.dma_start(dst[0 : src.shape[0], :], src[:])
            else:
                nc.gpsimd.collective_compute(
                    "AllGather", ALU.bypass, replica_groups=[list(range(NCORE))],
                    ins=[src.opt()], outs=[dst.opt()])

        def tr128(dst, src_tile, eye, n=4):
            """transpose n [128-col] blocks of src into dst slices (via psum)."""
            for i in range(n):
                tp = ps_tr.tile([128, 128], src_tile.dtype, tag="ptr", name="ptr")
                nc.tensor.transpose(tp[:], src_tile[:, 128 * i : 128 * (i + 1)], eye)
                nc.vector.tensor_copy(dst[:, 128 * i : 128 * (i + 1)], tp[:])

        def mm4(lhsT_tile, rhs_tiles):
            pm = ps_mm.tile([128, D], F32, tag="pmm", name="pmm")
            for kk in range(4):
                nc.tensor.matmul(pm[:], lhsT_tile[:, 128 * kk : 128 * (kk + 1)], rhs_tiles[kk][:],
                                 start=(kk == 0), stop=(kk == 3))
            return pm

        # ---------- A0: x load, rms sums, transposes, chunk sums ----------
        for rt in range(NT):
            xt = dmaload(par, [128, D], F32, xs_d[128 * rt : 128 * (rt + 1), :], "xt")
            scr = par.tile([128, D], F32, tag="scr", name="scr")
            nc.vector.tensor_tensor(scr[:], xt[:], xt[:], ALU.mult)
            nc.vector.tensor_reduce(ss_all[:, rt : rt + 1], scr[:], AX.X, ALU.add)
            for i in range(4):
                tp = ps_tr.tile([128, 128], F32, tag="ptr", name="ptr")
                nc.tensor.transpose(tp[:], xt[:, 128 * i : 128 * (i + 1)], eye32[:])
                nc.vector.tensor_copy(xtT16[rt][:, 128 * i : 128 * (i + 1)], tp[:])
            for dt in range(4):
                nc.vector.tensor_reduce(
                    cmp_[dt][:, 4 * rt : 4 * rt + 4],
                    xtT16[rt][:, 128 * dt : 128 * (dt + 1)].rearrange("p (t j) -> p t j", j=32),
                    AX.X, ALU.add)
        # rinv for all tiles in one shot (one Sqrt table load)
        nrm_all = pers.tile([128, NT], F32, tag="nrm_all", name="nrm_all")
        nc.scalar.activation(nrm_all[:], ss_all[:], AF.Sqrt, scale=1.0 / D, bias=eps_t[:])
        nc.vector.reciprocal(rinv_all[:], nrm_all[:])

        for dt in range(4):
            nc.sync.dma_start(cm_sh[128 * dt : 128 * (dt + 1), :], cmp_[dt][:])
        allgather(cm_sh, cmG)

        # ---------- gates (replicated) ----------
        cmT = [pa.tile([128, B * NSTEP], BF16, tag=f"cmT{i}", name=f"cmT{i}") for i in range(4)]
        for dt in range(4):
            for b in range(B):
                tmp0 = par.tile([128, NSTEP], F32, tag="cmg0", name="cmg0")
                tmp1 = par.tile([128, NSTEP], F32, tag="cmg1", name="cmg1")
                nc.sync.dma_start(tmp0[:], cmG[D * (2 * b) + 128 * dt : D * (2 * b) + 128 * (dt + 1), :])
                nc.sync.dma_start(tmp1[:], cmG[D * (2 * b + 1) + 128 * dt : D * (2 * b + 1) + 128 * (dt + 1), :])
                nc.vector.tensor_tensor(
                    cmT[dt][:, NSTEP * b : NSTEP * (b + 1)], tmp0[:], tmp1[:], ALU.add)

        wg = [[dmaload(pa, [128, D], BF16, wgT_d[g][128 * i : 128 * (i + 1), :], f"wg{g}_{i}")
               for i in range(4)] for g in range(3)]
        bg = [[dmaload(pa, [128, 1], F32, bg_d[g][128 * i : 128 * (i + 1), :], f"bg{g}_{i}")
               for i in range(4)] for g in range(3)]
        ones16 = pa.tile([128, 1], BF16, tag="ones", name="ones")
        nc.vector.memset(ones16[:], 1.0)
        gvec = []
        for g in range(3):
            gT = []
            for et in range(4):
                pm = ps_mm.tile([128, B * NSTEP], F32, tag="pmm", name="pmm")
                for dt in range(4):
                    nc.tensor.matmul(
                        pm[:], wg[g][dt][:, 128 * et : 128 * (et + 1)], cmT[dt][:],
                        start=(dt == 0), stop=(dt == 3))
                gt = pa.tile([128, B * NSTEP], BF16, tag=f"gT{et}", name=f"gT{et}")
                nc.scalar.activation(gt[:], pm[:], AF.Sigmoid, bias=bg[g][et][:])
                gT.append(gt)
            ps_s = ps_gr.tile([1, B * NSTEP], F32, tag="pgrad", name="pgrad")
            for et in range(4):
                nc.tensor.matmul(ps_s[:], ones16[:], gT[et][:], start=(et == 0), stop=(et == 3))
            svf = pa.tile([1, B * NSTEP], F32, tag=f"svf{g}", name=f"svf{g}")
            nc.vector.tensor_copy(svf[:], ps_s[:])
            sv = pa.tile([1, NSTEP], F32, tag=f"sv{g}", name=f"sv{g}")
            t01 = pa.tile([1, NSTEP], F32, tag="t01", name="t01")
            nc.vector.tensor_tensor(t01[:], svf[:, 0:NSTEP], svf[:, NSTEP : 2 * NSTEP], ALU.add)
            nc.vector.tensor_tensor(sv[:], svf[:, 2 * NSTEP : 3 * NSTEP], svf[:, 3 * NSTEP :], ALU.add)
            nc.vector.tensor_tensor(sv[:], t01[:], sv[:], ALU.add)
            gvec.append(sv)

        SM = 1.0 / (D * B)
        fin = pa.tile([1, 3 * NSTEP], F32, tag="fin", name="fin")
        nc.vector.tensor_scalar(fin[:, 0:NSTEP], gvec[0][:], -DEC * SM, 1.0, ALU.mult, ALU.add)
        nc.vector.tensor_scalar(fin[:, NSTEP : 2 * NSTEP], gvec[2][:], MOM * SM, None, ALU.mult)
        nc.vector.tensor_scalar(fin[:, 2 * NSTEP :], gvec[1][:], -LR * INV_N * SM, None, ALU.mult)
        nc.sync.dma_start(gbounce[:], fin[:])
        finb = pa.tile([1, 3 * NSTEP], F32, tag="finb", name="finb")
        nc.sync.dma_start(finb[:], gbounce[:])
        nc.gpsimd.partition_broadcast(avc[:], finb[:, 0:NSTEP])
        nc.gpsimd.partition_broadcast(evc[:], finb[:, NSTEP : 2 * NSTEP])
        nc.gpsimd.partition_broadcast(cvc[:], finb[:, 2 * NSTEP :])

        # ---------- projections in groups, each followed by its AllGather ----------
        mb = [dmaload(pa, [128, D], BF16, mb_d[128 * i : 128 * (i + 1), :], f"mb{i}") for i in range(4)]
        wkT = [dmaload(pa, [128, D], BF16, wkT_d[128 * i : 128 * (i + 1), :], f"wkT{i}") for i in range(4)]
        wvT = [dmaload(pa, [128, D], BF16, wvT_d[128 * i : 128 * (i + 1), :], f"wvT{i}") for i in range(4)]
        wqT = [dmaload(pa, [128, D], BF16, wqT_d[128 * i : 128 * (i + 1), :], f"wqT{i}") for i in range(4)]

        def kTview(g, j):
            """[p, i, j] view of pkg[g] kT region at 128-col block j."""
            return pkg[g][0:D, :].rearrange("(i p) c -> p i c", p=128)[:, :, 128 * j : 128 * (j + 1)]

        for g in range(NGRP if not skip_proj else 0):
            kp = []
            qp = []
            kqss = par.tile([128, 2 * TPG], F32, tag="kqss", name="kqss")
            for j in range(TPG):
                rt = TPG * g + j
                xT = xtT16[rt]
                rinv = rinv_all[:, rt : rt + 1]
                # t1 = rinv * (x @ Mb)   (no activation here)
                pm = mm4(xT, mb)
                t1sb = par.tile([128, D], BF16, tag="t1sb", name="t1sb")
                nc.vector.tensor_scalar(t1sb[:], pm[:], rinv, None, ALU.mult)
                t1sT = par.tile([128, D], BF16, tag="t1sT", name="t1sT")
                tr128(t1sT, t1sb, eye16[:])
                # k pre-norm: silu(t1 @ WkT) via sigmoid product
                pmk = mm4(t1sT, wkT)
                sgk = par.tile([128, D], BF16, tag="sgk", name="sgk")
                nc.scalar.activation(sgk[:], pmk[:], AF.Sigmoid)
                kpj = parg.tile([128, D], BF16, tag=f"kp{j}", name=f"kp{j}")
                nc.vector.tensor_tensor(kpj[:], pmk[:], sgk[:], ALU.mult)
                kp.append(kpj)
                scrk = par.tile([128, D], F32, tag="scr", name="scr")
                nc.vector.tensor_tensor(scrk[:], kpj[:], kpj[:], ALU.mult)
                nc.vector.tensor_reduce(kqss[:, j : j + 1], scrk[:], AX.X, ALU.add)
                # v = silu(rinv * (x @ WvT))
                pmv = mm4(xT, wvT)
                sgv = par.tile([128, D], BF16, tag="sgv", name="sgv")
                nc.scalar.activation(sgv[:], pmv[:], AF.Sigmoid, scale=rinv)
                vt_ = par.tile([128, D], BF16, tag="vtile", name="vtile")
                nc.vector.scalar_tensor_tensor(vt_[:], pmv[:], rinv, sgv[:], ALU.mult, ALU.mult)
                nc.sync.dma_start(pkg[g][2 * D + 128 * j : 2 * D + 128 * (j + 1), :], vt_[:])
                # q pre-norm
                pmq = mm4(xT, wqT)
                sgq = par.tile([128, D], BF16, tag="sgq", name="sgq")
                nc.scalar.activation(sgq[:], pmq[:], AF.Sigmoid, scale=rinv)
                qpj = parg.tile([128, D], BF16, tag=f"qp{j}", name=f"qp{j}")
                nc.vector.scalar_tensor_tensor(qpj[:], pmq[:], rinv, sgq[:], ALU.mult, ALU.mult)
                qp.append(qpj)
                scrq = par.tile([128, D], F32, tag="scr", name="scr")
                nc.vector.tensor_tensor(scrq[:], qpj[:], qpj[:], ALU.mult)
                nc.vector.tensor_reduce(kqss[:, TPG + j : TPG + j + 1], scrq[:], AX.X, ALU.add)
            # batched l2 rsqrt for the whole group (one table load)
            knr = par.tile([128, 2 * TPG], F32, tag="knr", name="knr")
            nc.scalar.activation(knr[:], kqss[:], AF.Sqrt)
            krs = par.tile([128, 2 * TPG], F32, tag="krs", name="krs")
            nc.vector.reciprocal(krs[:], knr[:])
            for j in range(TPG):
                rt = TPG * g + j
                kn = par.tile([128, D], BF16, tag="kn", name="kn")
                nc.vector.tensor_scalar(kn[:], kp[j][:], krs[:, j : j + 1], None, ALU.mult)
                nc.sync.dma_start(pkg[g][D + 128 * j : D + 128 * (j + 1), :], kn[:])
                knT = par.tile([128, D], BF16, tag="knT", name="knT")
                tr128(knT, kn, eye16[:])
                nc.sync.dma_start(kTview(g, j), knT[:].rearrange("p (i j) -> p i j", j=128))
                qn = par.tile([128, D], BF16, tag="qn", name="qn")
                nc.vector.tensor_scalar(qn[:], qp[j][:], krs[:, TPG + j : TPG + j + 1], None, ALU.mult)
                qnT = par.tile([128, D], BF16, tag="qnT", name="qnT")
                tr128(qnT, qn, eye16[:])
                nc.sync.dma_start(
                    qT_d[:].rearrange("(i p) c -> p i c", p=128)[:, :, 128 * rt : 128 * (rt + 1)],
                    qnT[:].rearrange("p (i j) -> p i j", j=128))
            allgather(pkg[g], pkgG[g])

        # ---------- the scan ----------
        for t in range(n_steps):
            g, j = t // 16, t % 16
            jsl = slice(32 * j, 32 * (j + 1))
            tsl = slice(32 * t, 32 * (t + 1))
            last = t == n_steps - 1

            qtT = rot3.tile([128, 128], BF16, tag="qtT", name="qtT")
            nc.sync.dma_start(
                qtT[:].rearrange("p (dt q) -> p dt q", q=32),
                qT_d[:, tsl].rearrange("(dt p) q -> p dt q", p=128))
            if not last:
                ktT = rot3.tile([128, 1024], BF16, tag="ktT", name="ktT")
                ktT_v = ktT[:].rearrange("p (dt q) -> p dt q", q=256)
                for c in range(NCORE):
                    src = pkgG[g][3 * D * c : 3 * D * c + D, jsl].rearrange("(dt p) q -> p dt q", p=128)
                    nc.sync.dma_start(ktT_v[:, :, 32 * c : 32 * (c + 1)], src)
                kt = [rot3.tile([128, D], BF16, tag=f"kt{rh}", name=f"kt{rh}") for rh in range(2)]
                vt = [rot3.tile([128, D], BF16, tag=f"vt{rh}", name=f"vt{rh}") for rh in range(2)]
                for c in range(NCORE):
                    rh, ro = c // 4, 32 * (c % 4)
                    kbase = 3 * D * c + D + 32 * j
                    vbase = 3 * D * c + 2 * D + 32 * j
                    nc.sync.dma_start(kt[rh][ro : ro + 32, :], pkgG[g][kbase : kbase + 32, :])
                    nc.sync.dma_start(vt[rh][ro : ro + 32, :], pkgG[g][vbase : vbase + 32, :])

            # P1/P2: retrieval for the local 32 rows (pre-update weights)
            p_hq = ps_mm.tile([32, D], F32, tag="pmm", name="pmm")
            for dt in range(4):
                nc.tensor.matmul(p_hq[:], qtT[:, 32 * dt : 32 * (dt + 1)], w0t16[dt][:],
                                 start=(dt == 0), stop=(dt == 3))

            # P3: h1 = k_t @ W0^T  (natural, [256, 512])
            if not last:
                p_h1 = [ps_mm.tile([128, D], F32, tag="pmm", name="pmm") for _ in range(2)]
                for rh in range(2):
                    for dt in range(4):
                        nc.tensor.matmul(p_h1[rh][:], ktT[:, 256 * dt + 128 * rh : 256 * dt + 128 * (rh + 1)],
                                         w0t16[dt][:], start=(dt == 0), stop=(dt == 3))

            # retrieval activation: sq = silu(hq)
            sq = rot.tile([32, D], BF16, tag="sq", name="sq")
            nc.scalar.activation(sq[:], p_hq[:], AF.Silu)
            p_sqT = ps_tr.tile([128, 128], BF16, tag="ptr", name="ptr")
            for it in range(4):
                nc.tensor.transpose(p_sqT[:, 32 * it : 32 * (it + 1)],
                                    sq[:, 128 * it : 128 * (it + 1)], eye16[0:32, 0:32])
            sqT = rot.tile([128, 128], BF16, tag="sqT", name="sqT")
            nc.vector.tensor_copy(sqT[:], p_sqT[:])
            p_ret = ps_mm.tile([32, D], F32, tag="pmm", name="pmm")
            for it in range(4):
                nc.tensor.matmul(p_ret[:], sqT[:, 32 * it : 32 * (it + 1)], w1t16[it][:],
                                 start=(it == 0), stop=(it == 3))
            rsb = rot.tile([32, D], BF16, tag="rsb", name="rsb")
            nc.vector.tensor_copy(rsb[:], p_ret[:])
            nc.sync.dma_start(ret_d[tsl, :], rsb[:])

            if last:
                break

            # silu + derivative on psum h1 (ACT tables; grouped -> 2 loads/step)
            a1 = [rot.tile([128, D], BF16, tag=f"a1_{rh}", name=f"a1_{rh}") for rh in range(2)]
            ds = [rot.tile([128, D], BF16, tag=f"ds_{rh}", name=f"ds_{rh}") for rh in range(2)]
            a1c = [rot.tile([128, D], BF16, tag=f"a1c_{rh}", name=f"a1c_{rh}") for rh in range(2)]
            for rh in range(2):
                nc.scalar.activation(a1[rh][:], p_h1[rh][:], AF.Silu)
            for rh in range(2):
                nc.scalar.activation(ds[rh][:], p_h1[rh][:], AF.Derivative_silu)
            for rh in range(2):
                nc.vector.tensor_scalar(a1c[rh][:], a1[rh][:], cvc[:, t : t + 1], None, ALU.mult)

            a1T = rot.tile([128, 1024], BF16, tag="a1T", name="a1T")
            for it in range(4):
                tp = ps_tr.tile([128, 256], BF16, tag="ptr", name="ptr")
                for rh in range(2):
                    nc.tensor.transpose(tp[:, 128 * rh : 128 * (rh + 1)],
                                        a1[rh][:, 128 * it : 128 * (it + 1)], eye16[:])
                nc.vector.tensor_copy(a1T[:, 256 * it : 256 * (it + 1)], tp[:])

            # P4: y
            p_y = [ps_mm.tile([128, D], F32, tag="pmm", name="pmm") for _ in range(2)]
            for rh in range(2):
                for it in range(4):
                    nc.tensor.matmul(p_y[rh][:], a1T[:, 256 * it + 128 * rh : 256 * it + 128 * (rh + 1)],
                                     w1t16[it][:], start=(it == 0), stop=(it == 3))
            dy = [rot.tile([128, D], BF16, tag=f"dy_{rh}", name=f"dy_{rh}") for rh in range(2)]
            for rh in range(2):
                nc.vector.tensor_tensor(dy[rh][:], p_y[rh][:], vt[rh][:], ALU.subtract)

            # P5: grad W1 -> momentum update in place (weight update deferred past P6)
            for ot in range(4):
                pg = ps_gr.tile([128, D], F32, tag="pgrad", name="pgrad")
                for rh in range(2):
                    nc.tensor.matmul(pg[:], dy[rh][:, 128 * ot : 128 * (ot + 1)], a1c[rh][:],
                                     start=(rh == 0), stop=(rh == 1))
                nc.vector.scalar_tensor_tensor(
                    m1[ot][:], m1[ot][:], evc[:, t : t + 1], pg[:], ALU.mult, ALU.add)

            dyT = rot.tile([128, 1024], BF16, tag="dyT", name="dyT")
            for ot in range(4):
                tp = ps_tr.tile([128, 256], BF16, tag="ptr", name="ptr")
                for rh in range(2):
                    nc.tensor.transpose(tp[:, 128 * rh : 128 * (rh + 1)],
                                        dy[rh][:, 128 * ot : 128 * (ot + 1)], eye16[:])
                nc.vector.tensor_copy(dyT[:, 256 * ot : 256 * (ot + 1)], tp[:])

            # P6: da1 (uses OLD w1n16)
            p_da = [ps_mm.tile([128, D], F32, tag="pmm", name="pmm") for _ in range(2)]
            for rh in range(2):
                for ot in range(4):
                    nc.tensor.matmul(p_da[rh][:], dyT[:, 256 * ot + 128 * rh : 256 * ot + 128 * (rh + 1)],
                                     w1n16[ot][:], start=(ot == 0), stop=(ot == 3))
            dh1 = [rot.tile([128, D], BF16, tag=f"dh1_{rh}", name=f"dh1_{rh}") for rh in range(2)]
            for rh in range(2):
                nc.vector.scalar_tensor_tensor(
                    dh1[rh][:], p_da[rh][:], cvc[:, t : t + 1], ds[rh][:], ALU.mult, ALU.mult)

            # P7: grad W0T -> momentum update in place
            for dt in range(4):
                pg = ps_gr.tile([128, D], F32, tag="pgrad", name="pgrad")
                for rh in range(2):
                    nc.tensor.matmul(pg[:], kt[rh][:, 128 * dt : 128 * (dt + 1)], dh1[rh][:],
                                     start=(rh == 0), stop=(rh == 1))
                nc.vector.scalar_tensor_tensor(
                    m0[dt][:], m0[dt][:], evc[:, t : t + 1], pg[:], ALU.mult, ALU.add)

            # deferred in-place weight updates (P6/P4/retrieval readers are done)
            for dt in range(4):
                nc.vector.scalar_tensor_tensor(
                    w0t16[dt][:], w0t32[dt][:], avc[:, t : t + 1], m0[dt][:], ALU.mult, ALU.add)
                nc.vector.scalar_tensor_tensor(
                    w0t32[dt][:], w0t32[dt][:], avc[:, t : t + 1], m0[dt][:], ALU.mult, ALU.add)
            for ot in range(4):
                nc.vector.scalar_tensor_tensor(
                    w1n16[ot][:], w1n32[ot][:], avc[:, t : t + 1], m1[ot][:], ALU.mult, ALU.add)
                nc.gpsimd.tensor_scalar(w1n32[ot][:], w1n32[ot][:], avc[:, t : t + 1], None, ALU.mult)
                nc.gpsimd.tensor_tensor(w1n32[ot][:], w1n32[ot][:], m1[ot][:], ALU.add)

            # W1T for next step (transpose the updated w1n16, in place)
            for it in range(4):
                tp = ps_tr.tile([128, D], BF16, tag="ptr", name="ptr")
                for ot in range(4):
                    nc.tensor.transpose(tp[:, 128 * ot : 128 * (ot + 1)],
                                        w1n16[ot][:, 128 * it : 128 * (it + 1)], eye16[:])
                nc.vector.tensor_copy(w1t16[it][:], tp[:])

        # ---------- phase C: out = ret @ Wout^T ----------
        woT = [dmaload(pa, [128, D], BF16, woT_d[128 * i : 128 * (i + 1), :], f"woT{i}") for i in range(4)]
        if skip_c:
            for rt in []:
                pass
        nt_c = NT if not skip_c else 1
        for rt in range(nt_c):
            rtile = dmaload(par, [128, D], BF16, ret_d[128 * rt : 128 * (rt + 1), :], "kn")
            rT = par.tile([128, D], BF16, tag="knT", name="knT")
            tr128(rT, rtile, eye16[:])
            pm = mm4(rT, woT)
            ot_ = par.tile([128, D], F32, tag="scr", name="scr")
            nc.vector.tensor_copy(ot_[:], pm[:])
            nc.sync.dma_start(out_d[128 * rt : 128 * (rt + 1), :], ot_[:])

    nc.compile()
    return nc


def kernel(x, M, mem_W, Wk, Wv, Wq, Wout, Wgd, bgd, Wgl, bgl, Wgm, bgm, gs, gr):
    import ml_dtypes
    from concourse.bass_utils import run_bass_kernel_spmd

    BF = ml_dtypes.bfloat16
    x = np.asarray(x, np.float32)
    M = np.asarray(M, np.float32)
    gs = np.asarray(gs, np.float32)
    gr = np.asarray(gr, np.float32)

    n_steps = int(__import__("os").environ.get("KERNEL_NSTEPS", NSTEP))
    key = n_steps
    if key not in _BUILT:
        _BUILT[key] = _build(n_steps)
    nc = _BUILT[key]

    shared = dict(
        WkT=np.ascontiguousarray(Wk.T).astype(BF),
        WvT=np.ascontiguousarray((Wv * gs[None, :]).T).astype(BF),
        WqT=np.ascontiguousarray((Wq * gr[None, :]).T).astype(BF),
        WoutT=np.ascontiguousarray(Wout.T).astype(BF),
        WgdT=np.ascontiguousarray(Wgd.T / C).astype(BF),
        WglT=np.ascontiguousarray(Wgl.T / C).astype(BF),
        WgmT=np.ascontiguousarray(Wgm.T / C).astype(BF),
        bgd=np.asarray(bgd, np.float32).reshape(D, 1),
        bgl=np.asarray(bgl, np.float32).reshape(D, 1),
        bgm=np.asarray(bgm, np.float32).reshape(D, 1),
        W0T32=np.ascontiguousarray(mem_W[0].T).astype(np.float32),
        W0T16=np.ascontiguousarray(mem_W[0].T).astype(BF),
        W1n32=np.ascontiguousarray(mem_W[1]).astype(np.float32),
        W1n16=np.ascontiguousarray(mem_W[1]).astype(BF),
        W1T16=np.ascontiguousarray(mem_W[1].T).astype(BF),
        EYE16=np.eye(128, dtype=BF),
        EYE32=np.eye(128, dtype=np.float32),
    )
    in_maps = []
    for c in range(NCORE):
        b, par = c // 2, c % 2
        m = dict(shared)
        m["xs"] = np.ascontiguousarray(x[b, par::2, :])
        m["Mb"] = (gs[:, None] * M[b]).astype(BF)
        in_maps.append(m)

    res = run_bass_kernel_spmd(nc, in_maps, list(range(NCORE)))
    out = np.empty((B, S, D), np.float32)
    for c in range(NCORE):
        b, par = c // 2, c % 2
        out[b, par::2, :] = res.results[c]["out"]
    return out


# revision 18
# speedup vs baseline: 1.6324x; 1.6324x over previous
"""ArchetypalNeuralMemory on 8 TRN2 NeuronCores (Bass/Tile).

Strategy (sharding_hint: data-parallel over B, replicate fast weights):
  - token sharding: core c owns batch b=c//2, parity p=c%2 -> tokens
    x[b, p::2, :]  (2048 tokens; 32 rows of every one of the 64 chunks).
  - projections (k,kT,v) computed on the local shard in 4 token-groups;
    each group is packed into one buffer and all-gathered so the gathers
    overlap later projection compute and the early scan steps.
  - the 64-step fast-weight scan is inherently serial, so it is
    REPLICATED on every core; only retrieval + output proj are sharded.
  - scalar engine runs ONLY Sigmoid in the scan (no ACT table reloads);
    silu and silu' are built from sigmoid on the vector engine.
  - momentum update is a vector scalar_tensor_tensor reading the grad
    PSUM directly (no eta*m matmul, no psum->sbuf momentum copy).
  - fp32 weight masters updated on the (otherwise idle) gpsimd engine;
    the bf16 shadows are produced first so the critical path never
    waits on the fp32 update.
"""

import sys

if "/opt/trn_rl_repo" not in sys.path:
    sys.path.insert(0, "/opt/trn_rl_repo")

import numpy as np

B, S, D = 4, 4096, 512
C = 64            # chunk length
NSTEP = 64        # chunks
NCORE = 8
TOK = 2048        # tokens per core
NT = TOK // 128   # 16 row tiles per core
NGRP = 4          # projection/AG groups
TPG = NT // NGRP  # tiles per group
LR, MOM, DEC = 0.1, 0.9, 0.01
EPS_RMS = 1.1920929e-07
INV_N = 2.0 / (B * C * D)

_BUILT = {}


def _build(n_steps=NSTEP):
    import concourse.bacc as bacc
    import concourse.mybir as mybir
    import concourse.tile as tile
    from contextlib import ExitStack
    import os as _os

    F32 = mybir.dt.float32
    BF16 = mybir.dt.bfloat16
    AF = mybir.ActivationFunctionType
    ALU = mybir.AluOpType
    AX = mybir.AxisListType

    nc = bacc.Bacc("TRN2", target_bir_lowering=False)
    P = nc.declare_dram_parameter

    xs_d = P("xs", [TOK, D], F32, isOutput=False)
    mb_d = P("Mb", [D, D], BF16, isOutput=False)        # gs-scaled M[b], [d,e]
    wkT_d = P("WkT", [D, D], BF16, isOutput=False)       # [e, e']
    wvT_d = P("WvT", [D, D], BF16, isOutput=False)       # gs-folded, [d, e]
    wqT_d = P("WqT", [D, D], BF16, isOutput=False)       # gr-folded, [d, e]
    woT_d = P("WoutT", [D, D], BF16, isOutput=False)     # [d, e]
    wgT_d = [P(n, [D, D], BF16, isOutput=False) for n in ("WgdT", "WglT", "WgmT")]
    bg_d = [P(n, [D, 1], F32, isOutput=False) for n in ("bgd", "bgl", "bgm")]
    w0t32_d = P("W0T32", [D, D], F32, isOutput=False)    # mem_W[0].T
    w0t16_d = P("W0T16", [D, D], BF16, isOutput=False)
    w1n32_d = P("W1n32", [D, D], F32, isOutput=False)    # mem_W[1]
    w1n16_d = P("W1n16", [D, D], BF16, isOutput=False)
    w1t16_d = P("W1T16", [D, D], BF16, isOutput=False)   # mem_W[1].T
    eye16_d = P("EYE16", [128, 128], BF16, isOutput=False)
    eye32_d = P("EYE32", [128, 128], F32, isOutput=False)
    out_d = P("out", [TOK, D], F32, isOutput=True)

    with tile.TileContext(nc) as tc, ExitStack() as ctx:
        dram = ctx.enter_context(tc.tile_pool(name="dram", bufs=1, space="DRAM"))
        ps_mm = ctx.enter_context(tc.tile_pool(name="ps_mm", bufs=4, space="PSUM"))
        ps_gr = ctx.enter_context(tc.tile_pool(name="ps_gr", bufs=2, space="PSUM"))
        ps_tr = ctx.enter_context(tc.tile_pool(name="ps_tr", bufs=2, space="PSUM"))
        pers = ctx.enter_context(tc.tile_pool(name="pers", bufs=1))
        pa = ctx.enter_context(tc.tile_pool(name="pa", bufs=1))
        par = ctx.enter_context(tc.tile_pool(name="par", bufs=2))
        parg = ctx.enter_context(tc.tile_pool(name="parg", bufs=1))
        rot = ctx.enter_context(tc.tile_pool(name="rot", bufs=2))
        rot3 = ctx.enter_context(tc.tile_pool(name="rot3", bufs=3))

        def dmaload(pool, shape, dt, src, tag):
            t = pool.tile(shape, dt, tag=tag, name=tag)
            nc.sync.dma_start(t[:], src)
            return t

        # ---------- persistent state ----------
        eye16 = dmaload(pers, [128, 128], BF16, eye16_d[:], "eye16")
        eye32 = dmaload(pers, [128, 128], F32, eye32_d[:], "eye32")
        # weight state: one wide tile per kind, column-block i holds rows 128i..128(i+1)
        def wide(tag, dt):
            return pers.tile([128, 4 * D], dt, tag=tag, name=tag)
        w0t32w, w1n32w = wide("w0t32w", F32), wide("w1n32w", F32)
        w0t16w, w1n16w, w1t16w = wide("w0t16w", BF16), wide("w1n16w", BF16), wide("w1t16w", BF16)
        for i in range(4):
            nc.sync.dma_start(w0t32w[:, D * i : D * (i + 1)], w0t32_d[128 * i : 128 * (i + 1), :])
            nc.sync.dma_start(w1n32w[:, D * i : D * (i + 1)], w1n32_d[128 * i : 128 * (i + 1), :])
            nc.sync.dma_start(w0t16w[:, D * i : D * (i + 1)], w0t16_d[128 * i : 128 * (i + 1), :])
            nc.sync.dma_start(w1n16w[:, D * i : D * (i + 1)], w1n16_d[128 * i : 128 * (i + 1), :])
            nc.sync.dma_start(w1t16w[:, D * i : D * (i + 1)], w1t16_d[128 * i : 128 * (i + 1), :])
        w0t16 = [w0t16w[:, D * i : D * (i + 1)] for i in range(4)]
        w1n16 = [w1n16w[:, D * i : D * (i + 1)] for i in range(4)]
        w1t16 = [w1t16w[:, D * i : D * (i + 1)] for i in range(4)]
        avc = pers.tile([128, NSTEP], F32, tag="avc", name="avc")
        evc = pers.tile([128, NSTEP], F32, tag="evc", name="evc")
        cvc = pers.tile([128, NSTEP], F32, tag="cvc", name="cvc")
        eps_t = pers.tile([128, 1], F32, tag="eps", name="eps")
        nc.vector.memset(eps_t[:], EPS_RMS)
        m0w, m1w = wide("m0w", BF16), wide("m1w", BF16)
        nc.vector.memset(m0w[:], 0.0)
        nc.vector.memset(m1w[:], 0.0)
        m0 = [m0w[:, D * i : D * (i + 1)] for i in range(4)]
        m1 = [m1w[:, D * i : D * (i + 1)] for i in range(4)]
        xtT16 = [pers.tile([128, D], BF16, tag=f"xtT{rt}", name=f"xtT{rt}") for rt in range(NT)]
        ss_all = pers.tile([128, NT], F32, tag="ss_all", name="ss_all")
        rinv_all = pers.tile([128, NT], F32, tag="rinv_all", name="rinv_all")
        cmp_ = [pers.tile([128, NSTEP], F32, tag=f"cmp{i}", name=f"cmp{i}") for i in range(4)]

        # dram buffers
        qT_d = dram.tile([D, TOK], BF16)
        ret_d = dram.tile([TOK, D], BF16)
        cm_sh = dram.tile([D, NSTEP], F32)
        cmG = dram.tile([D * NCORE, NSTEP], F32, addr_space="Shared")
        # packed per-group projection buffer: rows 0-511 kT, 512-1023 k, 1024-1535 v
        pkg = [dram.tile([3 * D, D], BF16, tag=f"pkg{g}", name=f"pkg{g}") for g in range(NGRP)]
        pkgG = [dram.tile([3 * D * NCORE, D], BF16, addr_space="Shared", tag=f"pkgG{g}", name=f"pkgG{g}")
                for g in range(NGRP)]
        gbounce = dram.tile([1, 3 * NSTEP], F32)

        skip_ag = _os.environ.get("KERNEL_SKIP_AG") == "1"
        skip_proj = _os.environ.get("KERNEL_SKIP_PROJ") == "1"
        skip_c = _os.environ.get("KERNEL_SKIP_PHASEC") == "1"

        def allgather(src, dst):
            if skip_ag:
                nc.gpsimd.dma_start(dst[0 : src.shape[0], :], src[:])
            else:
                nc.gpsimd.collective_compute(
                    "AllGather", ALU.bypass, replica_groups=[list(range(NCORE))],
                    ins=[src.opt()], outs=[dst.opt()])

        def tr128(dst, src_tile, eye, n=4):
            """transpose n [128-col] blocks of src into dst slices (via psum)."""
            for i in range(n):
                tp = ps_tr.tile([128, 128], src_tile.dtype, tag="ptr", name="ptr")
                nc.tensor.transpose(tp[:], src_tile[:, 128 * i : 128 * (i + 1)], eye)
                nc.vector.tensor_copy(dst[:, 128 * i : 128 * (i + 1)], tp[:])

        def mm4(lhsT_tile, rhs_tiles):
            pm = ps_mm.tile([128, D], F32, tag="pmm", name="pmm")
            for kk in range(4):
                nc.tensor.matmul(pm[:], lhsT_tile[:, 128 * kk : 128 * (kk + 1)], rhs_tiles[kk][:],
                                 start=(kk == 0), stop=(kk == 3))
            return pm

        # ---------- A0: x load, rms sums, transposes, chunk sums ----------
        for rt in range(NT):
            xt = dmaload(par, [128, D], F32, xs_d[128 * rt : 128 * (rt + 1), :], "xt")
            scr = par.tile([128, D], F32, tag="scr", name="scr")
            nc.vector.tensor_tensor(scr[:], xt[:], xt[:], ALU.mult)
            nc.vector.tensor_reduce(ss_all[:, rt : rt + 1], scr[:], AX.X, ALU.add)
            for i in range(4):
                tp = ps_tr.tile([128, 128], F32, tag="ptr", name="ptr")
                nc.tensor.transpose(tp[:], xt[:, 128 * i : 128 * (i + 1)], eye32[:])
                nc.vector.tensor_copy(xtT16[rt][:, 128 * i : 128 * (i + 1)], tp[:])
            for dt in range(4):
                nc.vector.tensor_reduce(
                    cmp_[dt][:, 4 * rt : 4 * rt + 4],
                    xtT16[rt][:, 128 * dt : 128 * (dt + 1)].rearrange("p (t j) -> p t j", j=32),
                    AX.X, ALU.add)
        # rinv for all tiles in one shot (one Sqrt table load)
        nrm_all = pers.tile([128, NT], F32, tag="nrm_all", name="nrm_all")
        nc.scalar.activation(nrm_all[:], ss_all[:], AF.Sqrt, scale=1.0 / D, bias=eps_t[:])
        nc.vector.reciprocal(rinv_all[:], nrm_all[:])

        for dt in range(4):
            nc.sync.dma_start(cm_sh[128 * dt : 128 * (dt + 1), :], cmp_[dt][:])
        allgather(cm_sh, cmG)

        # ---------- gates (replicated) ----------
        cmT = [pa.tile([128, B * NSTEP], BF16, tag=f"cmT{i}", name=f"cmT{i}") for i in range(4)]
        for dt in range(4):
            for b in range(B):
                tmp0 = par.tile([128, NSTEP], F32, tag="cmg0", name="cmg0")
                tmp1 = par.tile([128, NSTEP], F32, tag="cmg1", name="cmg1")
                nc.sync.dma_start(tmp0[:], cmG[D * (2 * b) + 128 * dt : D * (2 * b) + 128 * (dt + 1), :])
                nc.sync.dma_start(tmp1[:], cmG[D * (2 * b + 1) + 128 * dt : D * (2 * b + 1) + 128 * (dt + 1), :])
                nc.vector.tensor_tensor(
                    cmT[dt][:, NSTEP * b : NSTEP * (b + 1)], tmp0[:], tmp1[:], ALU.add)

        wg = [[dmaload(pa, [128, D], BF16, wgT_d[g][128 * i : 128 * (i + 1), :], f"wg{g}_{i}")
               for i in range(4)] for g in range(3)]
        bg = [[dmaload(pa, [128, 1], F32, bg_d[g][128 * i : 128 * (i + 1), :], f"bg{g}_{i}")
               for i in range(4)] for g in range(3)]
        ones16 = pa.tile([128, 1], BF16, tag="ones", name="ones")
        nc.vector.memset(ones16[:], 1.0)
        gvec = []
        for g in range(3):
            gT = []
            for et in range(4):
                pm = ps_mm.tile([128, B * NSTEP], F32, tag="pmm", name="pmm")
                for dt in range(4):
                    nc.tensor.matmul(
                        pm[:], wg[g][dt][:, 128 * et : 128 * (et + 1)], cmT[dt][:],
                        start=(dt == 0), stop=(dt == 3))
                gt = pa.tile([128, B * NSTEP], BF16, tag=f"gT{et}", name=f"gT{et}")
                nc.scalar.activation(gt[:], pm[:], AF.Sigmoid, bias=bg[g][et][:])
                gT.append(gt)
            ps_s = ps_gr.tile([1, B * NSTEP], F32, tag="pgrad", name="pgrad")
            for et in range(4):
                nc.tensor.matmul(ps_s[:], ones16[:], gT[et][:], start=(et == 0), stop=(et == 3))
            svf = pa.tile([1, B * NSTEP], F32, tag=f"svf{g}", name=f"svf{g}")
            nc.vector.tensor_copy(svf[:], ps_s[:])
            sv = pa.tile([1, NSTEP], F32, tag=f"sv{g}", name=f"sv{g}")
            t01 = pa.tile([1, NSTEP], F32, tag="t01", name="t01")
            nc.vector.tensor_tensor(t01[:], svf[:, 0:NSTEP], svf[:, NSTEP : 2 * NSTEP], ALU.add)
            nc.vector.tensor_tensor(sv[:], svf[:, 2 * NSTEP : 3 * NSTEP], svf[:, 3 * NSTEP :], ALU.add)
            nc.vector.tensor_tensor(sv[:], t01[:], sv[:], ALU.add)
            gvec.append(sv)

        SM = 1.0 / (D * B)
        fin = pa.tile([1, 3 * NSTEP], F32, tag="fin", name="fin")
        nc.vector.tensor_scalar(fin[:, 0:NSTEP], gvec[0][:], -DEC * SM, 1.0, ALU.mult, ALU.add)
        nc.vector.tensor_scalar(fin[:, NSTEP : 2 * NSTEP], gvec[2][:], MOM * SM, None, ALU.mult)
        nc.vector.tensor_scalar(fin[:, 2 * NSTEP :], gvec[1][:], -LR * INV_N * SM, None, ALU.mult)
        nc.sync.dma_start(gbounce[:], fin[:])
        finb = pa.tile([1, 3 * NSTEP], F32, tag="finb", name="finb")
        nc.sync.dma_start(finb[:], gbounce[:])
        nc.gpsimd.partition_broadcast(avc[:], finb[:, 0:NSTEP])
        nc.gpsimd.partition_broadcast(evc[:], finb[:, NSTEP : 2 * NSTEP])
        nc.gpsimd.partition_broadcast(cvc[:], finb[:, 2 * NSTEP :])

        # ---------- projections in groups, each followed by its AllGather ----------
        mb = [dmaload(pa, [128, D], BF16, mb_d[128 * i : 128 * (i + 1), :], f"mb{i}") for i in range(4)]
        wkT = [dmaload(pa, [128, D], BF16, wkT_d[128 * i : 128 * (i + 1), :], f"wkT{i}") for i in range(4)]
        wvT = [dmaload(pa, [128, D], BF16, wvT_d[128 * i : 128 * (i + 1), :], f"wvT{i}") for i in range(4)]
        wqT = [dmaload(pa, [128, D], BF16, wqT_d[128 * i : 128 * (i + 1), :], f"wqT{i}") for i in range(4)]

        def kTview(g, j):
            """[p, i, j] view of pkg[g] kT region at 128-col block j."""
            return pkg[g][0:D, :].rearrange("(i p) c -> p i c", p=128)[:, :, 128 * j : 128 * (j + 1)]

        for g in range(NGRP if not skip_proj else 0):
            kp = []
            qp = []
            kqss = par.tile([128, 2 * TPG], F32, tag="kqss", name="kqss")
            for j in range(TPG):
                rt = TPG * g + j
                xT = xtT16[rt]
                rinv = rinv_all[:, rt : rt + 1]
                # t1 = rinv * (x @ Mb)   (no activation here)
                pm = mm4(xT, mb)
                t1sb = par.tile([128, D], BF16, tag="t1sb", name="t1sb")
                nc.vector.tensor_scalar(t1sb[:], pm[:], rinv, None, ALU.mult)
                t1sT = par.tile([128, D], BF16, tag="t1sT", name="t1sT")
                tr128(t1sT, t1sb, eye16[:])
                # k pre-norm: silu(t1 @ WkT) via sigmoid product
                pmk = mm4(t1sT, wkT)
                sgk = par.tile([128, D], BF16, tag="sgk", name="sgk")
                nc.scalar.activation(sgk[:], pmk[:], AF.Sigmoid)
                kpj = parg.tile([128, D], BF16, tag=f"kp{j}", name=f"kp{j}")
                nc.vector.tensor_tensor(kpj[:], pmk[:], sgk[:], ALU.mult)
                kp.append(kpj)
                scrk = par.tile([128, D], F32, tag="scr", name="scr")
                nc.vector.tensor_tensor(scrk[:], kpj[:], kpj[:], ALU.mult)
                nc.vector.tensor_reduce(kqss[:, j : j + 1], scrk[:], AX.X, ALU.add)
                # v = silu(rinv * (x @ WvT))
                pmv = mm4(xT, wvT)
                sgv = par.tile([128, D], BF16, tag="sgv", name="sgv")
                nc.scalar.activation(sgv[:], pmv[:], AF.Sigmoid, scale=rinv)
                vt_ = par.tile([128, D], BF16, tag="vtile", name="vtile")
                nc.vector.scalar_tensor_tensor(vt_[:], pmv[:], rinv, sgv[:], ALU.mult, ALU.mult)
                nc.sync.dma_start(pkg[g][2 * D + 128 * j : 2 * D + 128 * (j + 1), :], vt_[:])
                # q pre-norm
                pmq = mm4(xT, wqT)
                sgq = par.tile([128, D], BF16, tag="sgq", name="sgq")
                nc.scalar.activation(sgq[:], pmq[:], AF.Sigmoid, scale=rinv)
                qpj = parg.tile([128, D], BF16, tag=f"qp{j}", name=f"qp{j}")
                nc.vector.scalar_tensor_tensor(qpj[:], pmq[:], rinv, sgq[:], ALU.mult, ALU.mult)
                qp.append(qpj)
                scrq = par.tile([128, D], F32, tag="scr", name="scr")
                nc.vector.tensor_tensor(scrq[:], qpj[:], qpj[:], ALU.mult)
                nc.vector.tensor_reduce(kqss[:, TPG + j : TPG + j + 1], scrq[:], AX.X, ALU.add)
            # batched l2 rsqrt for the whole group (one table load)
            knr = par.tile([128, 2 * TPG], F32, tag="knr", name="knr")
            nc.scalar.activation(knr[:], kqss[:], AF.Sqrt)
            krs = par.tile([128, 2 * TPG], F32, tag="krs", name="krs")
            nc.vector.reciprocal(krs[:], knr[:])
            for j in range(TPG):
                rt = TPG * g + j
                kn = par.tile([128, D], BF16, tag="kn", name="kn")
                nc.vector.tensor_scalar(kn[:], kp[j][:], krs[:, j : j + 1], None, ALU.mult)
                nc.sync.dma_start(pkg[g][D + 128 * j : D + 128 * (j + 1), :], kn[:])
                knT = par.tile([128, D], BF16, tag="knT", name="knT")
                tr128(knT, kn, eye16[:])
                nc.sync.dma_start(kTview(g, j), knT[:].rearrange("p (i j) -> p i j", j=128))
                qn = par.tile([128, D], BF16, tag="qn", name="qn")
                nc.vector.tensor_scalar(qn[:], qp[j][:], krs[:, TPG + j : TPG + j + 1], None, ALU.mult)
                qnT = par.tile([128, D], BF16, tag="qnT", name="qnT")
                tr128(qnT, qn, eye16[:])
                nc.sync.dma_start(
                    qT_d[:].rearrange("(i p) c -> p i c", p=128)[:, :, 128 * rt : 128 * (rt + 1)],
                    qnT[:].rearrange("p (i j) -> p i j", j=128))
            allgather(pkg[g], pkgG[g])

        # ---------- the scan ----------
        for t in range(n_steps):
            g, j = t // 16, t % 16
            jsl = slice(32 * j, 32 * (j + 1))
            tsl = slice(32 * t, 32 * (t + 1))
            last = t == n_steps - 1

            qtT = rot3.tile([128, 128], BF16, tag="qtT", name="qtT")
            nc.sync.dma_start(
                qtT[:].rearrange("p (dt q) -> p dt q", q=32),
                qT_d[:, tsl].rearrange("(dt p) q -> p dt q", p=128))
            if not last:
                ktT = rot3.tile([128, 1024], BF16, tag="ktT", name="ktT")
                ktT_v = ktT[:].rearrange("p (dt q) -> p dt q", q=256)
                for c in range(NCORE):
                    src = pkgG[g][3 * D * c : 3 * D * c + D, jsl].rearrange("(dt p) q -> p dt q", p=128)
                    nc.sync.dma_start(ktT_v[:, :, 32 * c : 32 * (c + 1)], src)
                kt = [rot3.tile([128, D], BF16, tag=f"kt{rh}", name=f"kt{rh}") for rh in range(2)]
                vt = [rot3.tile([128, D], BF16, tag=f"vt{rh}", name=f"vt{rh}") for rh in range(2)]
                for c in range(NCORE):
                    rh, ro = c // 4, 32 * (c % 4)
                    kbase = 3 * D * c + D + 32 * j
                    vbase = 3 * D * c + 2 * D + 32 * j
                    nc.sync.dma_start(kt[rh][ro : ro + 32, :], pkgG[g][kbase : kbase + 32, :])
                    nc.sync.dma_start(vt[rh][ro : ro + 32, :], pkgG[g][vbase : vbase + 32, :])

            # P1/P2: retrieval for the local 32 rows (pre-update weights)
            p_hq = ps_mm.tile([32, D], F32, tag="pmm", name="pmm")
            for dt in range(4):
                nc.tensor.matmul(p_hq[:], qtT[:, 32 * dt : 32 * (dt + 1)], w0t16[dt],
                                 start=(dt == 0), stop=(dt == 3))

            # P3: h1 = k_t @ W0^T  (natural, [256, 512])
            if not last:
                p_h1 = [ps_mm.tile([128, D], F32, tag="pmm", name="pmm") for _ in range(2)]
                for rh in range(2):
                    for dt in range(4):
                        nc.tensor.matmul(p_h1[rh][:], ktT[:, 256 * dt + 128 * rh : 256 * dt + 128 * (rh + 1)],
                                         w0t16[dt], start=(dt == 0), stop=(dt == 3))

            # retrieval activation: sq = silu(hq)
            sq = rot.tile([32, D], BF16, tag="sq", name="sq")
            nc.scalar.activation(sq[:], p_hq[:], AF.Silu)
            p_sqT = ps_tr.tile([128, 128], BF16, tag="ptr", name="ptr")
            for it in range(4):
                nc.tensor.transpose(p_sqT[:, 32 * it : 32 * (it + 1)],
                                    sq[:, 128 * it : 128 * (it + 1)], eye16[0:32, 0:32])
            sqT = rot.tile([128, 128], BF16, tag="sqT", name="sqT")
            nc.vector.tensor_copy(sqT[:], p_sqT[:])
            p_ret = ps_mm.tile([32, D], F32, tag="pmm", name="pmm")
            for it in range(4):
                nc.tensor.matmul(p_ret[:], sqT[:, 32 * it : 32 * (it + 1)], w1t16[it],
                                 start=(it == 0), stop=(it == 3))
            rsb = rot.tile([32, D], BF16, tag="rsb", name="rsb")
            nc.vector.tensor_copy(rsb[:], p_ret[:])
            nc.sync.dma_start(ret_d[tsl, :], rsb[:])

            if last:
                break

            # silu + derivative on psum h1 (ACT tables; grouped -> 2 loads/step)
            a1 = [rot.tile([128, D], BF16, tag=f"a1_{rh}", name=f"a1_{rh}") for rh in range(2)]
            ds = [rot.tile([128, D], BF16, tag=f"ds_{rh}", name=f"ds_{rh}") for rh in range(2)]
            a1c = [rot.tile([128, D], BF16, tag=f"a1c_{rh}", name=f"a1c_{rh}") for rh in range(2)]
            for rh in range(2):
                nc.scalar.activation(a1[rh][:], p_h1[rh][:], AF.Silu)
            for rh in range(2):
                nc.scalar.activation(ds[rh][:], p_h1[rh][:], AF.Derivative_silu)
            for rh in range(2):
                nc.vector.tensor_scalar(a1c[rh][:], a1[rh][:], cvc[:, t : t + 1], None, ALU.mult)

            a1T = rot.tile([128, 1024], BF16, tag="a1T", name="a1T")
            for it in range(4):
                tp = ps_tr.tile([128, 256], BF16, tag="ptr", name="ptr")
                for rh in range(2):
                    nc.tensor.transpose(tp[:, 128 * rh : 128 * (rh + 1)],
                                        a1[rh][:, 128 * it : 128 * (it + 1)], eye16[:])
                nc.vector.tensor_copy(a1T[:, 256 * it : 256 * (it + 1)], tp[:])

            # P4: y
            p_y = [ps_mm.tile([128, D], F32, tag="pmm", name="pmm") for _ in range(2)]
            for rh in range(2):
                for it in range(4):
                    nc.tensor.matmul(p_y[rh][:], a1T[:, 256 * it + 128 * rh : 256 * it + 128 * (rh + 1)],
                                     w1t16[it], start=(it == 0), stop=(it == 3))
            dy = [rot.tile([128, D], BF16, tag=f"dy_{rh}", name=f"dy_{rh}") for rh in range(2)]
            for rh in range(2):
                nc.vector.tensor_tensor(dy[rh][:], p_y[rh][:], vt[rh][:], ALU.subtract)

            # P5: grad W1 -> momentum update in place (weight update deferred past P6)
            for ot in range(4):
                pg = ps_gr.tile([128, D], F32, tag="pgrad", name="pgrad")
                for rh in range(2):
                    nc.tensor.matmul(pg[:], dy[rh][:, 128 * ot : 128 * (ot + 1)], a1c[rh][:],
                                     start=(rh == 0), stop=(rh == 1))
                nc.vector.scalar_tensor_tensor(
                    m1[ot], m1[ot], evc[:, t : t + 1], pg[:], ALU.mult, ALU.add)

            dyT = rot.tile([128, 1024], BF16, tag="dyT", name="dyT")
            for ot in range(4):
                tp = ps_tr.tile([128, 256], BF16, tag="ptr", name="ptr")
                for rh in range(2):
                    nc.tensor.transpose(tp[:, 128 * rh : 128 * (rh + 1)],
                                        dy[rh][:, 128 * ot : 128 * (ot + 1)], eye16[:])
                nc.vector.tensor_copy(dyT[:, 256 * ot : 256 * (ot + 1)], tp[:])

            # P6: da1 (uses OLD w1n16)
            p_da = [ps_mm.tile([128, D], F32, tag="pmm", name="pmm") for _ in range(2)]
            for rh in range(2):
                for ot in range(4):
                    nc.tensor.matmul(p_da[rh][:], dyT[:, 256 * ot + 128 * rh : 256 * ot + 128 * (rh + 1)],
                                     w1n16[ot], start=(ot == 0), stop=(ot == 3))
            dh1 = [rot.tile([128, D], BF16, tag=f"dh1_{rh}", name=f"dh1_{rh}") for rh in range(2)]
            for rh in range(2):
                nc.vector.scalar_tensor_tensor(
                    dh1[rh][:], p_da[rh][:], cvc[:, t : t + 1], ds[rh][:], ALU.mult, ALU.mult)

            # P7: grad W0T -> momentum update in place
            for dt in range(4):
                pg = ps_gr.tile([128, D], F32, tag="pgrad", name="pgrad")
                for rh in range(2):
                    nc.tensor.matmul(pg[:], kt[rh][:, 128 * dt : 128 * (dt + 1)], dh1[rh][:],
                                     start=(rh == 0), stop=(rh == 1))
                nc.vector.scalar_tensor_tensor(
                    m0[dt], m0[dt], evc[:, t : t + 1], pg[:], ALU.mult, ALU.add)

            # deferred in-place weight updates (P6/P4/retrieval readers are done)
            nc.vector.scalar_tensor_tensor(
                w0t16w[:], w0t32w[:], avc[:, t : t + 1], m0w[:], ALU.mult, ALU.add)
            nc.vector.scalar_tensor_tensor(
                w0t32w[:], w0t32w[:], avc[:, t : t + 1], m0w[:], ALU.mult, ALU.add)
            nc.vector.scalar_tensor_tensor(
                w1n16w[:], w1n32w[:], avc[:, t : t + 1], m1w[:], ALU.mult, ALU.add)
            nc.vector.scalar_tensor_tensor(
                w1n32w[:], w1n32w[:], avc[:, t : t + 1], m1w[:], ALU.mult, ALU.add)

            # W1T for next step (transpose the updated w1n16, in place)
            for it in range(4):
                tp = ps_tr.tile([128, D], BF16, tag="ptr", name="ptr")
                for ot in range(4):
                    nc.tensor.transpose(tp[:, 128 * ot : 128 * (ot + 1)],
                                        w1n16w[:, D * ot + 128 * it : D * ot + 128 * (it + 1)], eye16[:])
                nc.vector.tensor_copy(w1t16[it], tp[:])

        # ---------- phase C: out = ret @ Wout^T ----------
        woT = [dmaload(pa, [128, D], BF16, woT_d[128 * i : 128 * (i + 1), :], f"woT{i}") for i in range(4)]
        if skip_c:
            for rt in []:
                pass
        nt_c = NT if not skip_c else 1
        for rt in range(nt_c):
            rtile = dmaload(par, [128, D], BF16, ret_d[128 * rt : 128 * (rt + 1), :], "kn")
            rT = par.tile([128, D], BF16, tag="knT", name="knT")
            tr128(rT, rtile, eye16[:])
            pm = mm4(rT, woT)
            ot_ = par.tile([128, D], F32, tag="scr", name="scr")
            nc.vector.tensor_copy(ot_[:], pm[:])
            nc.sync.dma_start(out_d[128 * rt : 128 * (rt + 1), :], ot_[:])

    nc.compile()
    return nc


def kernel(x, M, mem_W, Wk, Wv, Wq, Wout, Wgd, bgd, Wgl, bgl, Wgm, bgm, gs, gr):
    import ml_dtypes
    from concourse.bass_utils import run_bass_kernel_spmd

    BF = ml_dtypes.bfloat16
    x = np.asarray(x, np.float32)
    M = np.asarray(M, np.float32)
    gs = np.asarray(gs, np.float32)
    gr = np.asarray(gr, np.float32)

    n_steps = int(__import__("os").environ.get("KERNEL_NSTEPS", NSTEP))
    key = n_steps
    if key not in _BUILT:
        _BUILT[key] = _build(n_steps)
    nc = _BUILT[key]

    shared = dict(
        WkT=np.ascontiguousarray(Wk.T).astype(BF),
        WvT=np.ascontiguousarray((Wv * gs[None, :]).T).astype(BF),
        WqT=np.ascontiguousarray((Wq * gr[None, :]).T).astype(BF),
        WoutT=np.ascontiguousarray(Wout.T).astype(BF),
        WgdT=np.ascontiguousarray(Wgd.T / C).astype(BF),
        WglT=np.ascontiguousarray(Wgl.T / C).astype(BF),
        WgmT=np.ascontiguousarray(Wgm.T / C).astype(BF),
        bgd=np.asarray(bgd, np.float32).reshape(D, 1),
        bgl=np.asarray(bgl, np.float32).reshape(D, 1),
        bgm=np.asarray(bgm, np.float32).reshape(D, 1),
        W0T32=np.ascontiguousarray(mem_W[0].T).astype(np.float32),
        W0T16=np.ascontiguousarray(mem_W[0].T).astype(BF),
        W1n32=np.ascontiguousarray(mem_W[1]).astype(np.float32),
        W1n16=np.ascontiguousarray(mem_W[1]).astype(BF),
        W1T16=np.ascontiguousarray(mem_W[1].T).astype(BF),
        EYE16=np.eye(128, dtype=BF),
        EYE32=np.eye(128, dtype=np.float32),
    )
    in_maps = []
    for c in range(NCORE):
        b, par = c // 2, c % 2
        m = dict(shared)
        m["xs"] = np.ascontiguousarray(x[b, par::2, :])
        m["Mb"] = (gs[:, None] * M[b]).astype(BF)
        in_maps.append(m)

    res = run_bass_kernel_spmd(nc, in_maps, list(range(NCORE)))
    out = np.empty((B, S, D), np.float32)
    for c in range(NCORE):
        b, par = c // 2, c % 2
        out[b, par::2, :] = res.results[c]["out"]
    return out


# revision 19
# speedup vs baseline: 1.7553x; 1.0753x over previous
"""ArchetypalNeuralMemory on 8 TRN2 NeuronCores (Bass/Tile).

Strategy (sharding_hint: data-parallel over B, replicate fast weights):
  - token sharding: core c owns batch b=c//2, parity p=c%2 -> tokens
    x[b, p::2, :]  (2048 tokens; 32 rows of every one of the 64 chunks).
  - projections (kT,k,v) are computed in 4 token-groups, each packed into
    one buffer and all-gathered; groups 1-3 are emitted INTERLEAVED with
    early scan steps so the gathers and projection compute hide under the
    scan (engine queues are FIFO in program order).
  - the 64-step fast-weight scan is inherently serial, so it is
    REPLICATED on every core; only retrieval + output proj are sharded,
    and the output projection is interleaved into the scan tail.
  - fast-weight state per step: momentum m (bf16, from grad PSUM via one
    scalar_tensor_tensor), bf16 weight shadows updated by a cheap bf16
    chain w16 = a*w16 + m, and fp32 masters that absorb a bf16 momentum
    accumulator only every 4 steps (off the critical path, resync cast
    into the chain at block boundaries). Numerics validated in numpy:
    rel_err ~6e-3 vs the 2e-2 gate.
  - scalar engine runs only Silu/D_Silu (2 table loads/step); all state
    math on the vector engine using wide [128,2048] tiles (one op per
    weight kind instead of four).
"""

import sys

if "/opt/trn_rl_repo" not in sys.path:
    sys.path.insert(0, "/opt/trn_rl_repo")

import numpy as np

B, S, D = 4, 4096, 512
C = 64            # chunk length
NSTEP = 64        # chunks
NCORE = 8
TOK = 2048        # tokens per core
NT = TOK // 128   # 16 row tiles per core
NGRP = 4          # projection/AG groups
TPG = NT // NGRP  # tiles per group
LR, MOM, DEC = 0.1, 0.9, 0.01
EPS_RMS = 1.1920929e-07
INV_N = 2.0 / (B * C * D)

_BUILT = {}


def _build(n_steps=NSTEP):
    import concourse.bacc as bacc
    import concourse.mybir as mybir
    import concourse.tile as tile
    from contextlib import ExitStack
    import os as _os

    F32 = mybir.dt.float32
    BF16 = mybir.dt.bfloat16
    AF = mybir.ActivationFunctionType
    ALU = mybir.AluOpType
    AX = mybir.AxisListType

    nc = bacc.Bacc("TRN2", target_bir_lowering=False)
    P = nc.declare_dram_parameter

    xs_d = P("xs", [TOK, D], F32, isOutput=False)
    mb_d = P("Mb", [D, D], BF16, isOutput=False)        # gs-scaled M[b], [d,e]
    wkT_d = P("WkT", [D, D], BF16, isOutput=False)       # [e, e']
    wvT_d = P("WvT", [D, D], BF16, isOutput=False)       # gs-folded, [d, e]
    wqT_d = P("WqT", [D, D], BF16, isOutput=False)       # gr-folded, [d, e]
    woT_d = P("WoutT", [D, D], BF16, isOutput=False)     # [d, e]
    wgT_d = [P(n, [D, D], BF16, isOutput=False) for n in ("WgdT", "WglT", "WgmT")]
    bg_d = [P(n, [D, 1], F32, isOutput=False) for n in ("bgd", "bgl", "bgm")]
    w0t32_d = P("W0T32", [D, D], F32, isOutput=False)    # mem_W[0].T
    w0t16_d = P("W0T16", [D, D], BF16, isOutput=False)
    w1n32_d = P("W1n32", [D, D], F32, isOutput=False)    # mem_W[1]
    w1n16_d = P("W1n16", [D, D], BF16, isOutput=False)
    w1t16_d = P("W1T16", [D, D], BF16, isOutput=False)   # mem_W[1].T
    eye16_d = P("EYE16", [128, 128], BF16, isOutput=False)
    eye32_d = P("EYE32", [128, 128], F32, isOutput=False)
    out_d = P("out", [TOK, D], F32, isOutput=True)

    with tile.TileContext(nc) as tc, ExitStack() as ctx:
        dram = ctx.enter_context(tc.tile_pool(name="dram", bufs=1, space="DRAM"))
        ps_mm = ctx.enter_context(tc.tile_pool(name="ps_mm", bufs=4, space="PSUM"))
        ps_gr = ctx.enter_context(tc.tile_pool(name="ps_gr", bufs=2, space="PSUM"))
        ps_tr = ctx.enter_context(tc.tile_pool(name="ps_tr", bufs=2, space="PSUM"))
        pers = ctx.enter_context(tc.tile_pool(name="pers", bufs=1))
        pa = ctx.enter_context(tc.tile_pool(name="pa", bufs=1))
        par = ctx.enter_context(tc.tile_pool(name="par", bufs=2))
        parg = ctx.enter_context(tc.tile_pool(name="parg", bufs=1))
        rot = ctx.enter_context(tc.tile_pool(name="rot", bufs=2))
        rot3 = ctx.enter_context(tc.tile_pool(name="rot3", bufs=2))

        def dmaload(pool, shape, dt, src, tag):
            t = pool.tile(shape, dt, tag=tag, name=tag)
            nc.sync.dma_start(t[:], src)
            return t

        # ---------- persistent state ----------
        eye16 = dmaload(pers, [128, 128], BF16, eye16_d[:], "eye16")
        eye32 = dmaload(pers, [128, 128], F32, eye32_d[:], "eye32")

        def wide(tag, dt):
            return pers.tile([128, 4 * D], dt, tag=tag, name=tag)

        w0t32w, w1n32w = wide("w0t32w", F32), wide("w1n32w", F32)
        w0t16w, w1n16w, w1t16w = wide("w0t16w", BF16), wide("w1n16w", BF16), wide("w1t16w", BF16)
        acc0w, acc1w = wide("acc0w", BF16), wide("acc1w", BF16)
        m0w, m1w = wide("m0w", BF16), wide("m1w", BF16)
        for i in range(4):
            nc.sync.dma_start(w0t32w[:, D * i : D * (i + 1)], w0t32_d[128 * i : 128 * (i + 1), :])
            nc.sync.dma_start(w1n32w[:, D * i : D * (i + 1)], w1n32_d[128 * i : 128 * (i + 1), :])
            nc.sync.dma_start(w0t16w[:, D * i : D * (i + 1)], w0t16_d[128 * i : 128 * (i + 1), :])
            nc.sync.dma_start(w1n16w[:, D * i : D * (i + 1)], w1n16_d[128 * i : 128 * (i + 1), :])
            nc.sync.dma_start(w1t16w[:, D * i : D * (i + 1)], w1t16_d[128 * i : 128 * (i + 1), :])
        nc.vector.memset(m0w[:], 0.0)
        nc.vector.memset(m1w[:], 0.0)
        nc.vector.memset(acc0w[:], 0.0)
        nc.vector.memset(acc1w[:], 0.0)
        w0t16 = [w0t16w[:, D * i : D * (i + 1)] for i in range(4)]
        w1n16 = [w1n16w[:, D * i : D * (i + 1)] for i in range(4)]
        w1t16 = [w1t16w[:, D * i : D * (i + 1)] for i in range(4)]
        m0 = [m0w[:, D * i : D * (i + 1)] for i in range(4)]
        m1 = [m1w[:, D * i : D * (i + 1)] for i in range(4)]

        avc = pers.tile([128, NSTEP], F32, tag="avc", name="avc")
        evc = pers.tile([128, NSTEP], F32, tag="evc", name="evc")
        cvc = pers.tile([128, NSTEP], F32, tag="cvc", name="cvc")
        av4 = pers.tile([128, NSTEP], F32, tag="av4", name="av4")
        eps_t = pers.tile([128, 1], F32, tag="eps", name="eps")
        nc.vector.memset(eps_t[:], EPS_RMS)
        xtT16 = [pers.tile([128, D], BF16, tag=f"xtT{rt}", name=f"xtT{rt}") for rt in range(NT)]
        ss_all = pers.tile([128, NT], F32, tag="ss_all", name="ss_all")
        rinv_all = pers.tile([128, NT], F32, tag="rinv_all", name="rinv_all")
        nrm_all = pers.tile([128, NT], F32, tag="nrm_all", name="nrm_all")
        cmp_ = [pers.tile([128, NSTEP], F32, tag=f"cmp{i}", name=f"cmp{i}") for i in range(4)]

        # dram buffers
        qT_d = dram.tile([D, TOK], BF16)
        ret_d = dram.tile([TOK, D], BF16)
        cm_sh = dram.tile([D, NSTEP], F32)
        cmG = dram.tile([D * NCORE, NSTEP], F32, addr_space="Shared")
        # packed per-group projection buffer: rows 0-511 kT, 512-1023 k, 1024-1535 v
        pkg = [dram.tile([3 * D, D], BF16, tag=f"pkg{g}", name=f"pkg{g}") for g in range(NGRP)]
        pkgG = [dram.tile([3 * D * NCORE, D], BF16, addr_space="Shared", tag=f"pkgG{g}", name=f"pkgG{g}")
                for g in range(NGRP)]
        gbounce = dram.tile([1, 4 * NSTEP], F32)

        skip_ag = _os.environ.get("KERNEL_SKIP_AG") == "1"

        def allgather(src, dst):
            if skip_ag:
                nc.gpsimd.dma_start(dst[0 : src.shape[0], :], src[:])
            else:
                nc.gpsimd.collective_compute(
                    "AllGather", ALU.bypass, replica_groups=[list(range(NCORE))],
                    ins=[src.opt()], outs=[dst.opt()])

        def tr128(dst, src_tile, eye, n=4):
            for i in range(n):
                tp = ps_tr.tile([128, 128], src_tile.dtype, tag="ptr", name="ptr")
                nc.tensor.transpose(tp[:], src_tile[:, 128 * i : 128 * (i + 1)], eye)
                nc.vector.tensor_copy(dst[:, 128 * i : 128 * (i + 1)], tp[:])

        def mm4(lhsT_tile, rhs_tiles):
            pm = ps_mm.tile([128, D], F32, tag="pmm", name="pmm")
            for kk in range(4):
                nc.tensor.matmul(pm[:], lhsT_tile[:, 128 * kk : 128 * (kk + 1)], rhs_tiles[kk][:],
                                 start=(kk == 0), stop=(kk == 3))
            return pm

        # ---------- A0: x load, rms sums, transposes, chunk sums ----------
        for rt in range(NT):
            xt = dmaload(par, [128, D], F32, xs_d[128 * rt : 128 * (rt + 1), :], "xt")
            scr = par.tile([128, D], F32, tag="scr", name="scr")
            nc.vector.tensor_tensor(scr[:], xt[:], xt[:], ALU.mult)
            nc.vector.tensor_reduce(ss_all[:, rt : rt + 1], scr[:], AX.X, ALU.add)
            for i in range(4):
                tp = ps_tr.tile([128, 128], F32, tag="ptr", name="ptr")
                nc.tensor.transpose(tp[:], xt[:, 128 * i : 128 * (i + 1)], eye32[:])
                nc.vector.tensor_copy(xtT16[rt][:, 128 * i : 128 * (i + 1)], tp[:])
            for dt in range(4):
                nc.vector.tensor_reduce(
                    cmp_[dt][:, 4 * rt : 4 * rt + 4],
                    xtT16[rt][:, 128 * dt : 128 * (dt + 1)].rearrange("p (t j) -> p t j", j=32),
                    AX.X, ALU.add)
        nc.scalar.activation(nrm_all[:], ss_all[:], AF.Sqrt, scale=1.0 / D, bias=eps_t[:])
        nc.vector.reciprocal(rinv_all[:], nrm_all[:])

        for dt in range(4):
            nc.sync.dma_start(cm_sh[128 * dt : 128 * (dt + 1), :], cmp_[dt][:])
        allgather(cm_sh, cmG)

        # ---------- gates (replicated) ----------
        cmT = [pa.tile([128, B * NSTEP], BF16, tag=f"cmT{i}", name=f"cmT{i}") for i in range(4)]
        for dt in range(4):
            for b in range(B):
                tmp0 = par.tile([128, NSTEP], F32, tag="cmg0", name="cmg0")
                tmp1 = par.tile([128, NSTEP], F32, tag="cmg1", name="cmg1")
                nc.sync.dma_start(tmp0[:], cmG[D * (2 * b) + 128 * dt : D * (2 * b) + 128 * (dt + 1), :])
                nc.sync.dma_start(tmp1[:], cmG[D * (2 * b + 1) + 128 * dt : D * (2 * b + 1) + 128 * (dt + 1), :])
                nc.vector.tensor_tensor(
                    cmT[dt][:, NSTEP * b : NSTEP * (b + 1)], tmp0[:], tmp1[:], ALU.add)

        wg = [[dmaload(pa, [128, D], BF16, wgT_d[g][128 * i : 128 * (i + 1), :], f"wg{g}_{i}")
               for i in range(4)] for g in range(3)]
        bg = [[dmaload(pa, [128, 1], F32, bg_d[g][128 * i : 128 * (i + 1), :], f"bg{g}_{i}")
               for i in range(4)] for g in range(3)]
        ones16 = pa.tile([128, 1], BF16, tag="ones", name="ones")
        nc.vector.memset(ones16[:], 1.0)
        gvec = []
        for g in range(3):
            gT = []
            for et in range(4):
                pm = ps_mm.tile([128, B * NSTEP], F32, tag="pmm", name="pmm")
                for dt in range(4):
                    nc.tensor.matmul(
                        pm[:], wg[g][dt][:, 128 * et : 128 * (et + 1)], cmT[dt][:],
                        start=(dt == 0), stop=(dt == 3))
                gt = pa.tile([128, B * NSTEP], BF16, tag=f"gT{et}", name=f"gT{et}")
                nc.scalar.activation(gt[:], pm[:], AF.Sigmoid, bias=bg[g][et][:])
                gT.append(gt)
            ps_s = ps_gr.tile([1, B * NSTEP], F32, tag="pgrad", name="pgrad")
            for et in range(4):
                nc.tensor.matmul(ps_s[:], ones16[:], gT[et][:], start=(et == 0), stop=(et == 3))
            svf = pa.tile([1, B * NSTEP], F32, tag=f"svf{g}", name=f"svf{g}")
            nc.vector.tensor_copy(svf[:], ps_s[:])
            sv = pa.tile([1, NSTEP], F32, tag=f"sv{g}", name=f"sv{g}")
            t01 = pa.tile([1, NSTEP], F32, tag="t01", name="t01")
            nc.vector.tensor_tensor(t01[:], svf[:, 0:NSTEP], svf[:, NSTEP : 2 * NSTEP], ALU.add)
            nc.vector.tensor_tensor(sv[:], svf[:, 2 * NSTEP : 3 * NSTEP], svf[:, 3 * NSTEP :], ALU.add)
            nc.vector.tensor_tensor(sv[:], t01[:], sv[:], ALU.add)
            gvec.append(sv)

        SM = 1.0 / (D * B)
        fin = pa.tile([1, 4 * NSTEP], F32, tag="fin", name="fin")
        nc.vector.tensor_scalar(fin[:, 0:NSTEP], gvec[0][:], -DEC * SM, 1.0, ALU.mult, ALU.add)
        nc.vector.tensor_scalar(fin[:, NSTEP : 2 * NSTEP], gvec[2][:], MOM * SM, None, ALU.mult)
        nc.vector.tensor_scalar(fin[:, 2 * NSTEP : 3 * NSTEP], gvec[1][:], -LR * INV_N * SM, None, ALU.mult)
        # av4[t] = prod of avc[t-3..t] (used at block boundaries; consumed at t+1)
        p2 = pa.tile([1, NSTEP], F32, tag="p2", name="p2")
        nc.vector.memset(p2[:], 1.0)
        nc.vector.tensor_tensor(p2[:, 1:NSTEP], fin[:, 1:NSTEP], fin[:, 0 : NSTEP - 1], ALU.mult)
        nc.vector.memset(fin[:, 3 * NSTEP : 4 * NSTEP], 1.0)
        nc.vector.tensor_tensor(
            fin[:, 3 * NSTEP + 3 : 4 * NSTEP], p2[:, 3:NSTEP], p2[:, 1 : NSTEP - 2], ALU.mult)
        nc.sync.dma_start(gbounce[:], fin[:])
        finb = pa.tile([1, 4 * NSTEP], F32, tag="finb", name="finb")
        nc.sync.dma_start(finb[:], gbounce[:])
        nc.gpsimd.partition_broadcast(avc[:], finb[:, 0:NSTEP])
        nc.gpsimd.partition_broadcast(evc[:], finb[:, NSTEP : 2 * NSTEP])
        nc.gpsimd.partition_broadcast(cvc[:], finb[:, 2 * NSTEP : 3 * NSTEP])
        nc.gpsimd.partition_broadcast(av4[:], finb[:, 3 * NSTEP :])

        # ---------- projection emission helpers ----------
        mb = [dmaload(pa, [128, D], BF16, mb_d[128 * i : 128 * (i + 1), :], f"mb{i}") for i in range(4)]
        wkT = [dmaload(pa, [128, D], BF16, wkT_d[128 * i : 128 * (i + 1), :], f"wkT{i}") for i in range(4)]
        wvT = [dmaload(pa, [128, D], BF16, wvT_d[128 * i : 128 * (i + 1), :], f"wvT{i}") for i in range(4)]
        wqT = [dmaload(pa, [128, D], BF16, wqT_d[128 * i : 128 * (i + 1), :], f"wqT{i}") for i in range(4)]
        woT = [dmaload(pa, [128, D], BF16, woT_d[128 * i : 128 * (i + 1), :], f"woT{i}") for i in range(4)]

        def kTview(g, j):
            return pkg[g][0:D, :].rearrange("(i p) c -> p i c", p=128)[:, :, 128 * j : 128 * (j + 1)]

        grp_state = {}

        def proj_tile(g, j):
            kp, qp, kqss = grp_state.setdefault(
                g, ([], [], pa.tile([128, 2 * TPG], F32, tag=f"kqss{g}", name=f"kqss{g}")))
            rt = TPG * g + j
            xT = xtT16[rt]
            rinv = rinv_all[:, rt : rt + 1]
            pm = mm4(xT, mb)
            t1sb = par.tile([128, D], BF16, tag="t1sb", name="t1sb")
            nc.vector.tensor_scalar(t1sb[:], pm[:], rinv, None, ALU.mult)
            t1sT = par.tile([128, D], BF16, tag="t1sT", name="t1sT")
            tr128(t1sT, t1sb, eye16[:])
            pmk = mm4(t1sT, wkT)
            sgk = par.tile([128, D], BF16, tag="sgk", name="sgk")
            nc.scalar.activation(sgk[:], pmk[:], AF.Sigmoid)
            kpj = parg.tile([128, D], BF16, tag=f"kp{j}", name=f"kp{j}")
            nc.vector.tensor_tensor(kpj[:], pmk[:], sgk[:], ALU.mult)
            kp.append(kpj)
            scrk = par.tile([128, D], F32, tag="scr", name="scr")
            nc.vector.tensor_tensor(scrk[:], kpj[:], kpj[:], ALU.mult)
            nc.vector.tensor_reduce(kqss[:, j : j + 1], scrk[:], AX.X, ALU.add)
            pmv = mm4(xT, wvT)
            sgv = par.tile([128, D], BF16, tag="sgv", name="sgv")
            nc.scalar.activation(sgv[:], pmv[:], AF.Sigmoid, scale=rinv)
            vt_ = par.tile([128, D], BF16, tag="vtile", name="vtile")
            nc.vector.scalar_tensor_tensor(vt_[:], pmv[:], rinv, sgv[:], ALU.mult, ALU.mult)
            nc.sync.dma_start(pkg[g][2 * D + 128 * j : 2 * D + 128 * (j + 1), :], vt_[:])
            pmq = mm4(xT, wqT)
            sgq = par.tile([128, D], BF16, tag="sgq", name="sgq")
            nc.scalar.activation(sgq[:], pmq[:], AF.Sigmoid, scale=rinv)
            qpj = parg.tile([128, D], BF16, tag=f"qp{j}", name=f"qp{j}")
            nc.vector.scalar_tensor_tensor(qpj[:], pmq[:], rinv, sgq[:], ALU.mult, ALU.mult)
            qp.append(qpj)
            scrq = par.tile([128, D], F32, tag="scr", name="scr")
            nc.vector.tensor_tensor(scrq[:], qpj[:], qpj[:], ALU.mult)
            nc.vector.tensor_reduce(kqss[:, TPG + j : TPG + j + 1], scrq[:], AX.X, ALU.add)

        def proj_tail(g):
            kp, qp, kqss = grp_state.pop(g)
            knr = par.tile([128, 2 * TPG], F32, tag="knr", name="knr")
            nc.scalar.activation(knr[:], kqss[:], AF.Sqrt)
            krs = par.tile([128, 2 * TPG], F32, tag="krs", name="krs")
            nc.vector.reciprocal(krs[:], knr[:])
            for j in range(TPG):
                rt = TPG * g + j
                kn = par.tile([128, D], BF16, tag="kn", name="kn")
                nc.vector.tensor_scalar(kn[:], kp[j][:], krs[:, j : j + 1], None, ALU.mult)
                nc.sync.dma_start(pkg[g][D + 128 * j : D + 128 * (j + 1), :], kn[:])
                knT = par.tile([128, D], BF16, tag="knT", name="knT")
                tr128(knT, kn, eye16[:])
                nc.sync.dma_start(kTview(g, j), knT[:].rearrange("p (i j) -> p i j", j=128))
                qn = par.tile([128, D], BF16, tag="qn", name="qn")
                nc.vector.tensor_scalar(qn[:], qp[j][:], krs[:, TPG + j : TPG + j + 1], None, ALU.mult)
                qnT = par.tile([128, D], BF16, tag="qnT", name="qnT")
                tr128(qnT, qn, eye16[:])
                nc.sync.dma_start(
                    qT_d[:].rearrange("(i p) c -> p i c", p=128)[:, :, 128 * rt : 128 * (rt + 1)],
                    qnT[:].rearrange("p (i j) -> p i j", j=128))
            allgather(pkg[g], pkgG[g])

        def c_tile(rt):
            rtile = dmaload(par, [128, D], BF16, ret_d[128 * rt : 128 * (rt + 1), :], "kn")
            rT = par.tile([128, D], BF16, tag="knT", name="knT")
            tr128(rT, rtile, eye16[:])
            pm = mm4(rT, woT)
            ot_ = par.tile([128, D], F32, tag="scr", name="scr")
            nc.vector.tensor_copy(ot_[:], pm[:])
            nc.sync.dma_start(out_d[128 * rt : 128 * (rt + 1), :], ot_[:])

        # group 0 upfront
        for j in range(TPG):
            proj_tile(0, j)
        proj_tail(0)

        # ---------- the scan ----------
        for t in range(n_steps):
            g, j = t // 16, t % 16
            jsl = slice(32 * j, 32 * (j + 1))
            tsl = slice(32 * t, 32 * (t + 1))
            last = t == n_steps - 1

            qtT = rot3.tile([128, 128], BF16, tag="qtT", name="qtT")
            nc.sync.dma_start(
                qtT[:].rearrange("p (dt q) -> p dt q", q=32),
                qT_d[:, tsl].rearrange("(dt p) q -> p dt q", p=128))
            if not last:
                ktT = rot3.tile([128, 1024], BF16, tag="ktT", name="ktT")
                ktT_v = ktT[:].rearrange("p (dt q) -> p dt q", q=256)
                for c in range(NCORE):
                    src = pkgG[g][3 * D * c : 3 * D * c + D, jsl].rearrange("(dt p) q -> p dt q", p=128)
                    nc.sync.dma_start(ktT_v[:, :, 32 * c : 32 * (c + 1)], src)
                kt = [rot3.tile([128, D], BF16, tag=f"kt{rh}", name=f"kt{rh}") for rh in range(2)]
                vt = [rot3.tile([128, D], BF16, tag=f"vt{rh}", name=f"vt{rh}") for rh in range(2)]
                for c in range(NCORE):
                    rh, ro = c // 4, 32 * (c % 4)
                    kbase = 3 * D * c + D + 32 * j
                    vbase = 3 * D * c + 2 * D + 32 * j
                    nc.sync.dma_start(kt[rh][ro : ro + 32, :], pkgG[g][kbase : kbase + 32, :])
                    nc.sync.dma_start(vt[rh][ro : ro + 32, :], pkgG[g][vbase : vbase + 32, :])

            # P1/P2: retrieval for the local 32 rows (pre-update weights)
            p_hq = ps_mm.tile([32, D], F32, tag="pmm", name="pmm")
            for dt in range(4):
                nc.tensor.matmul(p_hq[:], qtT[:, 32 * dt : 32 * (dt + 1)], w0t16[dt],
                                 start=(dt == 0), stop=(dt == 3))

            # P3: h1 = k_t @ W0^T  (natural, [256, 512])
            if not last:
                p_h1 = [ps_mm.tile([128, D], F32, tag="pmm", name="pmm") for _ in range(2)]
                for rh in range(2):
                    for dt in range(4):
                        nc.tensor.matmul(p_h1[rh][:], ktT[:, 256 * dt + 128 * rh : 256 * dt + 128 * (rh + 1)],
                                         w0t16[dt], start=(dt == 0), stop=(dt == 3))

            sq = rot.tile([32, D], BF16, tag="sq", name="sq")
            nc.scalar.activation(sq[:], p_hq[:], AF.Silu)
            p_sqT = ps_tr.tile([128, 128], BF16, tag="ptr", name="ptr")
            for it in range(4):
                nc.tensor.transpose(p_sqT[:, 32 * it : 32 * (it + 1)],
                                    sq[:, 128 * it : 128 * (it + 1)], eye16[0:32, 0:32])
            sqT = rot.tile([128, 128], BF16, tag="sqT", name="sqT")
            nc.vector.tensor_copy(sqT[:], p_sqT[:])
            p_ret = ps_mm.tile([32, D], F32, tag="pmm", name="pmm")
            for it in range(4):
                nc.tensor.matmul(p_ret[:], sqT[:, 32 * it : 32 * (it + 1)], w1t16[it],
                                 start=(it == 0), stop=(it == 3))
            rsb = rot.tile([32, D], BF16, tag="rsb", name="rsb")
            nc.vector.tensor_copy(rsb[:], p_ret[:])
            nc.sync.dma_start(ret_d[tsl, :], rsb[:])

            if not last:
                # block-boundary master absorb: ACC holds momenta through step t-1
                resync = t % 4 == 0 and t > 0
                if resync:
                    nc.vector.scalar_tensor_tensor(
                        w0t32w[:], w0t32w[:], av4[:, t - 1 : t], acc0w[:], ALU.mult, ALU.add)
                    nc.vector.scalar_tensor_tensor(
                        w1n32w[:], w1n32w[:], av4[:, t - 1 : t], acc1w[:], ALU.mult, ALU.add)

                a1 = [rot.tile([128, D], BF16, tag=f"a1_{rh}", name=f"a1_{rh}") for rh in range(2)]
                ds = [rot.tile([128, D], BF16, tag=f"ds_{rh}", name=f"ds_{rh}") for rh in range(2)]
                a1c = [rot.tile([128, D], BF16, tag=f"a1c_{rh}", name=f"a1c_{rh}") for rh in range(2)]
                for rh in range(2):
                    nc.scalar.activation(a1[rh][:], p_h1[rh][:], AF.Silu)
                for rh in range(2):
                    nc.scalar.activation(ds[rh][:], p_h1[rh][:], AF.Derivative_silu)
                for rh in range(2):
                    nc.vector.tensor_scalar(a1c[rh][:], a1[rh][:], cvc[:, t : t + 1], None, ALU.mult)

                a1T = rot.tile([128, 1024], BF16, tag="a1T", name="a1T")
                for half in range(2):
                    tp = ps_tr.tile([128, 512], BF16, tag="ptr", name="ptr")
                    for q in range(2):
                        it = 2 * half + q
                        for rh in range(2):
                            nc.tensor.transpose(tp[:, 256 * q + 128 * rh : 256 * q + 128 * (rh + 1)],
                                                a1[rh][:, 128 * it : 128 * (it + 1)], eye16[:])
                    nc.vector.tensor_copy(a1T[:, 512 * half : 512 * (half + 1)], tp[:])

                # P4: y
                p_y = [ps_mm.tile([128, D], F32, tag="pmm", name="pmm") for _ in range(2)]
                for rh in range(2):
                    for it in range(4):
                        nc.tensor.matmul(p_y[rh][:], a1T[:, 256 * it + 128 * rh : 256 * it + 128 * (rh + 1)],
                                         w1t16[it], start=(it == 0), stop=(it == 3))
                dy = [rot.tile([128, D], BF16, tag=f"dy_{rh}", name=f"dy_{rh}") for rh in range(2)]
                for rh in range(2):
                    nc.vector.tensor_tensor(dy[rh][:], p_y[rh][:], vt[rh][:], ALU.subtract)

                # P5: grad W1 -> momentum update in place
                for ot in range(4):
                    pg = ps_gr.tile([128, D], F32, tag="pgrad", name="pgrad")
                    for rh in range(2):
                        nc.tensor.matmul(pg[:], dy[rh][:, 128 * ot : 128 * (ot + 1)], a1c[rh][:],
                                         start=(rh == 0), stop=(rh == 1))
                    nc.vector.scalar_tensor_tensor(
                        m1[ot], m1[ot], evc[:, t : t + 1], pg[:], ALU.mult, ALU.add)

                dyT = rot.tile([128, 1024], BF16, tag="dyT", name="dyT")
                for half in range(2):
                    tp = ps_tr.tile([128, 512], BF16, tag="ptr", name="ptr")
                    for q in range(2):
                        ot = 2 * half + q
                        for rh in range(2):
                            nc.tensor.transpose(tp[:, 256 * q + 128 * rh : 256 * q + 128 * (rh + 1)],
                                                dy[rh][:, 128 * ot : 128 * (ot + 1)], eye16[:])
                    nc.vector.tensor_copy(dyT[:, 512 * half : 512 * (half + 1)], tp[:])

                # P6: da1 (uses OLD w1n16)
                p_da = [ps_mm.tile([128, D], F32, tag="pmm", name="pmm") for _ in range(2)]
                for rh in range(2):
                    for ot in range(4):
                        nc.tensor.matmul(p_da[rh][:], dyT[:, 256 * ot + 128 * rh : 256 * ot + 128 * (rh + 1)],
                                         w1n16[ot], start=(ot == 0), stop=(ot == 3))
                dh1 = [rot.tile([128, D], BF16, tag=f"dh1_{rh}", name=f"dh1_{rh}") for rh in range(2)]
                for rh in range(2):
                    nc.vector.scalar_tensor_tensor(
                        dh1[rh][:], p_da[rh][:], cvc[:, t : t + 1], ds[rh][:], ALU.mult, ALU.mult)

                # P7: grad W0T -> momentum update in place
                for dt in range(4):
                    pg = ps_gr.tile([128, D], F32, tag="pgrad", name="pgrad")
                    for rh in range(2):
                        nc.tensor.matmul(pg[:], kt[rh][:, 128 * dt : 128 * (dt + 1)], dh1[rh][:],
                                         start=(rh == 0), stop=(rh == 1))
                    nc.vector.scalar_tensor_tensor(
                        m0[dt], m0[dt], evc[:, t : t + 1], pg[:], ALU.mult, ALU.add)

                # shadow chain + accumulator (W0 first: next step's P3 needs it)
                if resync:
                    nc.vector.tensor_copy(w0t16w[:], w0t32w[:])
                    nc.vector.tensor_copy(acc0w[:], m0w[:])
                nc.vector.scalar_tensor_tensor(
                    w0t16w[:], w0t16w[:], avc[:, t : t + 1], m0w[:], ALU.mult, ALU.add)
                if not resync:
                    nc.vector.scalar_tensor_tensor(
                        acc0w[:], acc0w[:], avc[:, t : t + 1], m0w[:], ALU.mult, ALU.add)
                if resync:
                    nc.vector.tensor_copy(w1n16w[:], w1n32w[:])
                    nc.vector.tensor_copy(acc1w[:], m1w[:])
                nc.vector.scalar_tensor_tensor(
                    w1n16w[:], w1n16w[:], avc[:, t : t + 1], m1w[:], ALU.mult, ALU.add)
                if not resync:
                    nc.vector.scalar_tensor_tensor(
                        acc1w[:], acc1w[:], avc[:, t : t + 1], m1w[:], ALU.mult, ALU.add)

                # W1T for next step (transpose updated w1n16, in place)
                for it in range(4):
                    tp = ps_tr.tile([128, D], BF16, tag="ptr", name="ptr")
                    for ot in range(4):
                        nc.tensor.transpose(tp[:, 128 * ot : 128 * (ot + 1)],
                                            w1n16w[:, D * ot + 128 * it : D * ot + 128 * (it + 1)], eye16[:])
                    nc.vector.tensor_copy(w1t16[it], tp[:])

            # interleaved projection groups and output tiles
            if t in (0, 1, 2):
                proj_tile(1, t)
            elif t == 3:
                proj_tile(1, 3)
                proj_tail(1)
            elif t in (8, 10, 12):
                proj_tile(2, (t - 8) // 2)
            elif t == 14:
                proj_tile(2, 3)
                proj_tail(2)
            elif t in (20, 22, 24):
                proj_tile(3, (t - 20) // 2)
            elif t == 26:
                proj_tile(3, 3)
                proj_tail(3)
            elif 31 <= t <= 59 and (t - 31) % 2 == 0:
                c_tile((t - 31) // 2)

        for rt in range(15, NT):
            c_tile(rt)

    nc.compile()
    return nc


def kernel(x, M, mem_W, Wk, Wv, Wq, Wout, Wgd, bgd, Wgl, bgl, Wgm, bgm, gs, gr):
    import ml_dtypes
    from concourse.bass_utils import run_bass_kernel_spmd

    BF = ml_dtypes.bfloat16
    x = np.asarray(x, np.float32)
    M = np.asarray(M, np.float32)
    gs = np.asarray(gs, np.float32)
    gr = np.asarray(gr, np.float32)

    n_steps = int(__import__("os").environ.get("KERNEL_NSTEPS", NSTEP))
    key = n_steps
    if key not in _BUILT:
        _BUILT[key] = _build(n_steps)
    nc = _BUILT[key]

    shared = dict(
        WkT=np.ascontiguousarray(Wk.T).astype(BF),
        WvT=np.ascontiguousarray((Wv * gs[None, :]).T).astype(BF),
        WqT=np.ascontiguousarray((Wq * gr[None, :]).T).astype(BF),
        WoutT=np.ascontiguousarray(Wout.T).astype(BF),
        WgdT=np.ascontiguousarray(Wgd.T / C).astype(BF),
        WglT=np.ascontiguousarray(Wgl.T / C).astype(BF),
        WgmT=np.ascontiguousarray(Wgm.T / C).astype(BF),
        bgd=np.asarray(bgd, np.float32).reshape(D, 1),
        bgl=np.asarray(bgl, np.float32).reshape(D, 1),
        bgm=np.asarray(bgm, np.float32).reshape(D, 1),
        W0T32=np.ascontiguousarray(mem_W[0].T).astype(np.float32),
        W0T16=np.ascontiguousarray(mem_W[0].T).astype(BF),
        W1n32=np.ascontiguousarray(mem_W[1]).astype(np.float32),
        W1n16=np.ascontiguousarray(mem_W[1]).astype(BF),
        W1T16=np.ascontiguousarray(mem_W[1].T).astype(BF),
        EYE16=np.eye(128, dtype=BF),
        EYE32=np.eye(128, dtype=np.float32),
    )
    in_maps = []
    for c in range(NCORE):
        b, par = c // 2, c % 2
        m = dict(shared)
        m["xs"] = np.ascontiguousarray(x[b, par::2, :])
        m["Mb"] = (gs[:, None] * M[b]).astype(BF)
        in_maps.append(m)

    res = run_bass_kernel_spmd(nc, in_maps, list(range(NCORE)))
    out = np.empty((B, S, D), np.float32)
    for c in range(NCORE):
        b, par = c // 2, c % 2
        out[b, par::2, :] = res.results[c]["out"]
    return out
